# revision 1
# baseline (speedup 1.0000x reference)
"""Masked dot-product attention on 8 Trainium2 NeuronCores.

Strategy (per core): head-parallel sharding. B*H = 64 (batch, head) pairs are
split 8 per core; each core runs the full attention for its heads.

All layout transforms happen on the HOST (numpy) so the device only issues
plain contiguous DMAs:
  qT/kT:  [npairs, 128, S] bf16, head i of a pair on partitions 64i..64i+63,
          DK-major (already transposed).
  v1:     [nheads, 128, CH, 65] bf16, kj-within-chunk on partitions, with the
          ones column baked in (row dv=64 accumulates softmax denominators).
  maskT:  [n_kj, 128, S] bf16 keep-mask (1-mask), kj on partitions.

Per-head-pair pipeline (S=2048, DK=64), "S-transposed" layout so the PV
matmul needs no transpose of the huge exp matrix:
  S_T[kj, qi] = K @ Q^T        (PE, bf16, psum strips [128 kj, 2x512 qi])
  E_T = exp(S_T / sqrt(dk))    (ScalarE; strips live in split PSUM tiles:
                                a 2-slot pair tile exp'd in ONE merged N=2048
                                ACTIVATE + a solo tile -> amortizes the ~185ns
                                per-instruction init without cross-WARs)
  E_T *= maskT (keep 0/1)      (DVE tensor_tensor, bf16 2x mode)
  O_T[dv', qi] += V'[kj]^T E_T (PE accumulate over kj)
  O = (O_T^T)[:, :64] * recip(O_T^T[:, 64])   (PE transpose + DVE)

The QK/exp emission runs LAG groups ahead of the mask/PV/epilogue phase so
the PE queue always has the next QK pair in front of PV work that waits on
the DVE.
"""

import math

import numpy as np

import concourse.bass as bass
import concourse.mybir as mybir
import concourse.tile as tile
from concourse import bacc
from concourse.masks import make_identity

F32 = mybir.dt.float32
BF16 = mybir.dt.bfloat16
AF = mybir.ActivationFunctionType
ALU = mybir.AluOpType

N_CORES = 8


def build_attention_nc(nheads: int, S: int, DK: int, scale: float) -> bass.Bass:
    nc = bacc.Bacc("TRN2", target_bir_lowering=False, debug=False,
                   num_devices=N_CORES)

    DV1 = DK + 1          # V plus a ones column for softmax denominators
    n_kj = S // 128       # kj tiles per head
    QBLK = 512            # qi span of one O_T accumulator
    n_qblk = S // QBLK
    OC = QBLK // 128      # 128-row output chunks per block
    CH = S // 128         # 128-row chunks along seq
    npairs = nheads // 2
    assert nheads % 2 == 0

    qt_d = nc.dram_tensor("qT", [npairs, 128, S], BF16, kind="ExternalInput")
    kt_d = nc.dram_tensor("kT", [npairs, 128, S], BF16, kind="ExternalInput")
    v1_d = nc.dram_tensor("v1", [nheads, 128, CH, DV1], BF16,
                          kind="ExternalInput")
    m_d = nc.dram_tensor("maskT", [n_kj, 128, S], BF16, kind="ExternalInput")
    o_d = nc.dram_tensor("out", [nheads, S, DK], F32, kind="ExternalOutput")

    with tile.TileContext(nc) as tc:
        with (
            tc.tile_pool(name="consts", bufs=1) as consts,
            tc.tile_pool(name="maskp", bufs=1) as maskp,
            tc.tile_pool(name="qkT", bufs=3) as qkt,
            tc.tile_pool(name="vp", bufs=3) as vp,
            tc.tile_pool(name="ep", bufs=8) as ep,
            tc.tile_pool(name="outp", bufs=4) as outp,
            tc.tile_pool(name="small", bufs=4) as small,
            tc.tile_pool(name="ring", bufs=1, space="PSUM") as ringp,
            tc.tile_pool(name="opsum", bufs=2, space="PSUM") as opsum,
        ):
            ident_f = consts.tile([DV1, DV1], F32)
            make_identity(nc, ident_f)

            # ---- per-pair inputs: plain DMAs.
            qk_t = {}     # hp -> (qT, kT)
            v1s_all = {}  # hp -> [v1_h0, v1_h1]

            def emit_pair_loads(hp, eng):
                tts = []
                for name, src in (("q", qt_d), ("k", kt_d)):
                    tT = qkt.tile([128, S], BF16, tag=f"{name}T",
                                  name=f"{name}T_{hp}")
                    eng.dma_start(out=tT, in_=src[hp])
                    tts.append(tT)
                qk_t[hp] = tuple(tts)
                v1s = []
                for i in (0, 1):
                    v1 = vp.tile([128, CH, DV1], BF16, tag=f"v1_{i}",
                                 name=f"v1_{2 * hp + i}")
                    eng.dma_start(out=v1, in_=v1_d[2 * hp + i])
                    v1s.append(v1)
                v1s_all[hp] = v1s

            # pair 0/1 loads go FIRST so the compute pipeline starts ~3us in;
            # the mask strips follow (first kj strips first - they are needed
            # soonest by the PV phase).
            emit_pair_loads(0, nc.sync)
            if npairs > 1:
                emit_pair_loads(1, nc.gpsimd)

            maskT = maskp.tile([128, n_kj, S], BF16, tag="maskT", name="maskT")
            for kt in range(n_kj):
                eng = nc.sync if kt % 2 == 0 else nc.gpsimd
                eng.dma_start(out=maskT[:, kt, :], in_=m_d[kt])

            # ---- PSUM layout -----------------------------------------------
            # pairtile: 2 strip slots for the merged-exp pairs (4 banks),
            # solotile: 1 slot (2 banks) -> their WARs stay independent;
            # opsum: ps_o / ps_nat share one rotating 2-buf tag (2 banks).
            pairt = ringp.tile([128, 2, 2 * QBLK], F32, tag="pair",
                               name="pairt")
            solot = ringp.tile([128, 2 * QBLK], F32, tag="solo", name="solot")

            # ---- main loop --------------------------------------------------
            n_strips = npairs * n_qblk * n_kj

            def strip_info(s):
                hp = s // (n_qblk * n_kj)
                qb = (s // n_kj) % n_qblk
                kj = s % n_kj
                return hp, qb, kj

            ps_o = {}     # (hp, qb) -> [ps_o_h0, ps_o_h1]
            e_of = {}     # s -> (e_tile, col_base)

            def emit_qk(s):
                hp, qb, kj = strip_info(s)
                qT2, kT2 = qk_t[hp]
                slot = s % 3
                dst = pairt[:, slot, :] if slot < 2 else solot
                q0 = qb * QBLK
                for i in (0, 1):
                    nc.tensor.matmul(
                        dst[:, i * QBLK : (i + 1) * QBLK],
                        kT2[64 * i : 64 * i + DK, kj * 128 : (kj + 1) * 128],
                        qT2[64 * i : 64 * i + DK, q0 : q0 + QBLK],
                        start=True, stop=True,
                    )

            def emit_exp_merged(s):
                # strips s (slot 0) and s+1 (slot 1) in one N=2048 ACTIVATE
                e2 = ep.tile([128, 2, 2 * QBLK], BF16, tag="e2",
                             name=f"e2_{s}")
                nc.scalar.activation(e2, pairt, AF.Exp, scale=scale)
                e_of[s] = (e2, 0)
                e_of[s + 1] = (e2, 2 * QBLK)

            def emit_exp_solo(s):
                e1 = ep.tile([128, 2 * QBLK], BF16, tag="e1", name=f"e1_{s}")
                nc.scalar.activation(e1, solot, AF.Exp, scale=scale)
                e_of[s] = (e1, 0)

            def emit_mask(s):
                hp, qb, kj = strip_info(s)
                q0 = qb * QBLK
                e_t, base = e_of[s]
                ev = bass.AP(
                    tensor=e_t.tensor, offset=e_t.offset + base,
                    ap=[e_t.ap[0], [1, 2 * QBLK]],
                )
                msl = maskT[:, kj, q0 : q0 + QBLK]
                mdup = bass.AP(
                    tensor=msl.tensor, offset=msl.offset,
                    ap=[msl.ap[0], [0, 2], msl.ap[-1]],
                )
                nc.vector.tensor_mul(ev, ev, mdup)

            def emit_pv(s):
                hp, qb, kj = strip_info(s)
                e_t, base = e_of[s]
                for i in (0, 1):
                    ev = bass.AP(
                        tensor=e_t.tensor,
                        offset=e_t.offset + base + i * QBLK,
                        ap=[e_t.ap[0], [1, QBLK]],
                    )
                    nc.tensor.matmul(
                        ps_o[(hp, qb)][i],
                        v1s_all[hp][i][:, kj, :],
                        ev,
                        start=(kj == 0), stop=(kj == n_kj - 1),
                        skip_group_check=True,
                    )
                del e_of[s]

            def emit_output(hp, qb):
                q0 = qb * QBLK
                for i in (0, 1):
                    h = 2 * hp + i
                    ot_sb = outp.tile([DV1, QBLK], F32, tag="ot",
                                      name=f"ot_{h}_{qb}")
                    nc.vector.tensor_copy(ot_sb, ps_o[(hp, qb)][i])
                    ps_nat = opsum.tile([128, OC, DV1], F32, tag="o",
                                        name=f"ps_nat_{h}_{qb}")
                    for c in range(OC):
                        nc.tensor.transpose(
                            ps_nat[:, c, :],
                            ot_sb[:, c * 128 : (c + 1) * 128],
                            ident_f,
                        )
                    rec = small.tile([128, OC], F32, tag="rec",
                                     name=f"rec_{h}_{qb}")
                    nc.vector.reciprocal(rec, ps_nat[:, :, DK])
                    o_sb = outp.tile([128, OC, DK], F32, tag="osb",
                                     name=f"o_sb_{h}_{qb}")
                    rb = bass.AP(tensor=rec.tensor, offset=rec.offset,
                                 ap=[rec.ap[0], rec.ap[-1], [0, DK]])
                    nc.vector.tensor_mul(o_sb, ps_nat[:, :, 0:DK], rb)
                    nc.gpsimd.dma_start(
                        out=o_d[h, q0 : q0 + QBLK, :].rearrange(
                            "(c p) d -> p c d", p=128
                        ),
                        in_=o_sb,
                    )
                del ps_o[(hp, qb)]

            def ensure_ps_o(s):
                hp, qb, kj = strip_info(s)
                if kj == 0:
                    ps_o[(hp, qb)] = [
                        opsum.tile([DV1, QBLK], F32, tag="o",
                                   name=f"ps_o_{hp}_{qb}_{i}")
                        for i in (0, 1)
                    ]

            def post_strip(s):
                """mask+PV for strip s, epilogue & prefetch hooks."""
                hp, qb, kj = strip_info(s)
                ensure_ps_o(s)
                emit_mask(s)
                emit_pv(s)
                if kj == n_kj - 1:
                    emit_output(hp, qb)
                # prefetch two pairs ahead early in qb0 (pairs 0/1 are loaded
                # in the prologue)
                if hp + 2 < npairs and qb == 0 and kj == 2:
                    emit_pair_loads(hp + 2, nc.sync)

            # group strips by psum slot: slots (0,1) -> merged exp, slot 2 ->
            # solo. QK+exp emission runs LAG groups ahead of mask/PV/epilogue
            # so the PE queue always has the next QK pair in front of PV work
            # that waits on the DVE.
            groups = []
            s = 0
            while s < n_strips:
                if s % 3 == 0 and s + 1 < n_strips:
                    groups.append((s, s + 1))
                    s += 2
                else:
                    groups.append((s,))
                    s += 1

            LAG = 2
            pending = []
            for g in groups:
                for t in g:
                    emit_qk(t)
                if len(g) == 2:
                    emit_exp_merged(g[0])
                elif g[0] % 3 == 2:
                    emit_exp_solo(g[0])
                else:
                    # final unpaired strip landed on a pair slot
                    e1 = ep.tile([128, 2 * QBLK], BF16, tag="e1",
                                 name=f"e1_{g[0]}")
                    nc.scalar.activation(e1, pairt[:, g[0] % 3, :], AF.Exp,
                                         scale=scale)
                    e_of[g[0]] = (e1, 0)
                pending.append(g)
                if len(pending) > LAG:
                    for t in pending.pop(0):
                        post_strip(t)
            for g in pending:
                for t in g:
                    post_strip(t)

    nc.compile()
    return nc


_NC_CACHE: dict = {}


def _get_nc(nheads, S, DK, scale):
    key = (nheads, S, DK, scale)
    if key not in _NC_CACHE:
        _NC_CACHE[key] = build_attention_nc(nheads, S, DK, scale)
    return _NC_CACHE[key]


def make_in_maps(queries, keys, values, d_k, mask):
    """Host-side sharding + layout prep. Returns (in_maps, shape_info)."""
    import ml_dtypes

    BF = ml_dtypes.bfloat16
    B, H, S, DK = queries.shape
    BH = B * H
    assert BH % N_CORES == 0
    hpc = BH // N_CORES
    npairs = hpc // 2
    CH = S // 128
    n_kj = S // 128

    q = np.ascontiguousarray(queries.reshape(BH, S, DK)).astype(BF)
    k = np.ascontiguousarray(keys.reshape(BH, S, DK)).astype(BF)
    v = np.ascontiguousarray(values.reshape(BH, S, DK)).astype(BF)

    # qT/kT: [BH//2 pairs, 128, S] with head i of a pair on partitions
    # 64i..64i+63, DK-major.
    def to_pairT(x):
        # [BH, S, DK] -> [BH, DK, S] -> [BH//2, 2*DK, S]
        xt = x.transpose(0, 2, 1)
        return np.ascontiguousarray(xt.reshape(BH // 2, 2 * DK, S))

    qT = to_pairT(q)
    kT = to_pairT(k)

    # v1: [BH, 128, CH, DK+1] with ones column baked in.
    v1 = np.ones((BH, 128, CH, DK + 1), dtype=BF)
    v1[:, :, :, :DK] = v.reshape(BH, CH, 128, DK).transpose(0, 2, 1, 3)

    # maskT: [n_kj, 128, S] bf16 keep-mask (1 - mask), kj on partitions.
    mT = (1 - mask.reshape(S, S)).astype(BF).T  # [kj, qi]
    mT = np.ascontiguousarray(mT.reshape(n_kj, 128, S))

    in_maps = [
        {
            "qT": qT[c * npairs : (c + 1) * npairs],
            "kT": kT[c * npairs : (c + 1) * npairs],
            "v1": v1[c * hpc : (c + 1) * hpc],
            "maskT": mT,
        }
        for c in range(N_CORES)
    ]
    return in_maps, (B, H, S, DK, hpc)


def kernel(queries, keys, values, d_k, mask):
    from concourse.bass_utils import run_bass_kernel_spmd

    in_maps, (B, H, S, DK, hpc) = make_in_maps(queries, keys, values, d_k,
                                               mask)
    scale = 1.0 / math.sqrt(float(d_k))
    nc = _get_nc(hpc, S, DK, scale)

    res = run_bass_kernel_spmd(nc, in_maps, core_ids=list(range(N_CORES)))
    out = np.concatenate([r["out"] for r in res.results], axis=0)
    return out.reshape(B, H, S, DK).astype(queries.dtype)



# revision 2
# speedup vs baseline: 1.0203x; 1.0203x over previous
"""Masked dot-product attention on 8 Trainium2 NeuronCores.

Strategy (per core): head-parallel sharding. B*H = 64 (batch, head) pairs are
split 8 per core; each core runs the full attention for its heads.

All layout transforms happen on the HOST (numpy) so the device only issues
plain contiguous DMAs:
  qT/kT:  [npairs, 4, 128, 512] bf16, head i of a pair on partitions
          64i..64i+63, DK-major, chunked along S so compute can start as soon
          as the first 128KB chunk lands.
  v1:     [nheads, 128, CH, 65] bf16, kj-within-chunk on partitions, with the
          ones column baked in (row dv=64 accumulates softmax denominators).
  maskT:  [128, n_kj, S] bf16 keep-mask (1-mask), kj-within-tile on
          partitions.

Per-head-pair pipeline (S=2048, DK=64), "S-transposed" layout so the PV
matmul needs no transpose of the huge exp matrix:
  S_T[kj, qi] = K @ Q^T        (PE, bf16, psum strips [128 kj, 2x512 qi];
                                the two heads' K=64 matmuls run CONCURRENTLY
                                in distinct PE row groups)
  E_T = exp(S_T / sqrt(dk))    (ScalarE; pair tile exp'd in ONE merged N=2048
                                ACTIVATE + a solo tile -> amortizes the fixed
                                per-instruction cost without cross-WARs)
  E_T *= maskT (keep 0/1)      (DVE tensor_tensor, bf16 2x mode; merged over
                                3 strips [128,3072] via a 4D mask AP when the
                                strips share (hp,qb))
  O_T[dv', qi] += V'[kj]^T E_T (PE accumulate over kj)
Epilogue: the unnormalized O_T[65, 512] (64 value rows + denominator row) is
copied PSUM->SBUF (DVE) and DMA'd out as-is; the softmax division and the
[dv, qi] -> [qi, dv] transpose happen on the HOST. This removes all PE
transposes and DVE reciprocal/multiply work from the device.

The QK/exp emission runs LAG groups ahead of the mask/PV/epilogue phase so
the PE queue always has the next QK pair in front of PV work that waits on
the DVE.
"""

import math

import numpy as np

import concourse.bass as bass
import concourse.mybir as mybir
import concourse.tile as tile
from concourse import bacc

F32 = mybir.dt.float32
BF16 = mybir.dt.bfloat16
AF = mybir.ActivationFunctionType
ALU = mybir.AluOpType

N_CORES = 8


def build_attention_nc(nheads: int, S: int, DK: int, scale: float) -> bass.Bass:
    nc = bacc.Bacc("TRN2", target_bir_lowering=False, debug=False,
                   num_devices=N_CORES)

    DV1 = DK + 1          # V plus a ones column for softmax denominators
    n_kj = S // 128       # kj tiles per head
    QBLK = 512            # qi span of one O_T accumulator
    n_qblk = S // QBLK
    CH = S // 128         # 128-row chunks along seq
    NCHK = 4              # q/k S-chunks per head-pair
    CHK = S // NCHK       # columns per chunk (512)
    npairs = nheads // 2
    assert nheads % 2 == 0

    qt_d = nc.dram_tensor("qT", [npairs, NCHK, 128, CHK], BF16,
                          kind="ExternalInput")
    kt_d = nc.dram_tensor("kT", [npairs, NCHK, 128, CHK], BF16,
                          kind="ExternalInput")
    v1_d = nc.dram_tensor("v1", [nheads, 128, CH, DV1], BF16,
                          kind="ExternalInput")
    m_d = nc.dram_tensor("maskT", [128, n_kj, S], BF16, kind="ExternalInput")
    o_d = nc.dram_tensor("out", [nheads, n_qblk, DV1, QBLK], F32,
                         kind="ExternalOutput")

    with tile.TileContext(nc) as tc:
        with (
            tc.tile_pool(name="maskp", bufs=1) as maskp,
            tc.tile_pool(name="qkT", bufs=3) as qkt,
            tc.tile_pool(name="vp", bufs=3) as vp,
            tc.tile_pool(name="ep", bufs=8) as ep,
            tc.tile_pool(name="outp", bufs=4) as outp,
            tc.tile_pool(name="ring", bufs=1, space="PSUM") as ringp,
            tc.tile_pool(name="opsum", bufs=2, space="PSUM") as opsum,
        ):
            # ---- per-pair inputs: plain chunked DMAs.
            qk_t = {}     # hp -> (q chunk tiles, k chunk tiles)
            v1s_all = {}  # hp -> [v1_h0, v1_h1]

            def emit_pair_loads(hp, eng, veng):
                kts, qts = [], []
                # k chunk 0 + q chunk 0 first: they unblock the first QKs.
                for c in range(NCHK):
                    kc = qkt.tile([128, CHK], BF16, tag=f"ktc{c}",
                                  name=f"ktc{c}_{hp}")
                    eng.dma_start(out=kc, in_=kt_d[hp, c])
                    kts.append(kc)
                    qc = qkt.tile([128, CHK], BF16, tag=f"qtc{c}",
                                  name=f"qtc{c}_{hp}")
                    eng.dma_start(out=qc, in_=qt_d[hp, c])
                    qts.append(qc)
                qk_t[hp] = (qts, kts)
                v1s = []
                for i in (0, 1):
                    v1 = vp.tile([128, CH, DV1], BF16, tag=f"v1_{i}",
                                 name=f"v1_{2 * hp + i}")
                    veng.dma_start(out=v1, in_=v1_d[2 * hp + i])
                    v1s.append(v1)
                v1s_all[hp] = v1s

            # pair 0 q/k goes FIRST so the compute pipeline starts ~3us in;
            # v1 + mask strips follow (first kj strips first - they are needed
            # soonest by the mask/PV phase).
            emit_pair_loads(0, nc.sync, nc.gpsimd)

            maskT = maskp.tile([128, n_kj, S], BF16, tag="maskT", name="maskT")
            for kt in range(n_kj):
                eng = nc.gpsimd if kt < n_kj // 2 else nc.sync
                eng.dma_start(out=maskT[:, kt, :], in_=m_d[:, kt, :])
            if npairs > 1:
                emit_pair_loads(1, nc.sync, nc.gpsimd)

            # ---- PSUM layout -----------------------------------------------
            # pairt: 2 strip slots for the merged-exp pairs (4 banks),
            # solot: 1 slot (2 banks) -> their WARs stay independent;
            # opsum: ps_o tiles share one rotating 2-buf tag (2 banks).
            pairt = ringp.tile([128, 2, 2 * QBLK], F32, tag="pair",
                               name="pairt")
            solot = ringp.tile([128, 2 * QBLK], F32, tag="solo", name="solot")

            # ---- main loop --------------------------------------------------
            n_strips = npairs * n_qblk * n_kj

            def strip_info(s):
                hp = s // (n_qblk * n_kj)
                qb = (s // n_kj) % n_qblk
                kj = s % n_kj
                return hp, qb, kj

            ps_o = {}     # (hp, qb) -> [ps_o_h0, ps_o_h1]
            e_of = {}     # s -> (e3_tile, slot)

            def emit_qk(s):
                hp, qb, kj = strip_info(s)
                qts, kts = qk_t[hp]
                slot = s % 3
                dst = pairt[:, slot, :] if slot < 2 else solot
                kc = kts[kj // (n_kj // NCHK)]
                k0 = (kj % (n_kj // NCHK)) * 128
                qc = qts[qb * QBLK // CHK]
                q0 = (qb * QBLK) % CHK
                for i in (0, 1):
                    nc.tensor.matmul(
                        dst[:, i * QBLK : (i + 1) * QBLK],
                        kc[64 * i : 64 * i + DK, k0 : k0 + 128],
                        qc[64 * i : 64 * i + DK, q0 : q0 + QBLK],
                        start=True, stop=True,
                    )

            def get_e3(s):
                """e3 tile shared by the 3 strips of s's triple."""
                t0 = (s // 3) * 3
                if t0 not in e_of:
                    e_of[t0] = ep.tile([128, 3, 2 * QBLK], BF16, tag="e3",
                                       name=f"e3_{t0}")
                return e_of[t0]

            def emit_exp_pair(s):
                # strips s (slot 0) and s+1 (slot 1) in one N=2048 ACTIVATE
                e3 = get_e3(s)
                nc.scalar.activation(e3[:, 0:2, :], pairt, AF.Exp, scale=scale)

            def emit_exp_solo(s):
                e3 = get_e3(s)
                nc.scalar.activation(e3[:, 2, :], solot, AF.Exp, scale=scale)

            def emit_exp_tail(s):
                # final unpaired strip landed on a pair slot
                e3 = get_e3(s)
                nc.scalar.activation(e3[:, s % 3, :], pairt[:, s % 3, :],
                                     AF.Exp, scale=scale)

            def emit_mask_strip(s):
                """fallback: mask one strip [128, 1024] with dup'd mask."""
                hp, qb, kj = strip_info(s)
                e3 = get_e3(s)
                ev = e3[:, s % 3, :]
                q0 = qb * QBLK
                msl = maskT[:, kj, q0 : q0 + QBLK]
                mdup = bass.AP(
                    tensor=msl.tensor, offset=msl.offset,
                    ap=[msl.ap[0], [0, 2], [1, QBLK]],
                )
                nc.vector.tensor_mul(ev, ev, mdup)

            def emit_mask_triple(s0):
                """merged: mask strips s0..s0+2 in one [128, 3072] DVE op."""
                hp, qb, kj = strip_info(s0)
                e3 = e_of[s0]
                q0 = qb * QBLK
                msl = maskT[:, kj, q0 : q0 + QBLK]
                m4 = bass.AP(
                    tensor=msl.tensor, offset=msl.offset,
                    ap=[msl.ap[0], [S, 3], [0, 2], [1, QBLK]],
                )
                nc.vector.tensor_mul(e3, e3, m4)

            def emit_pv(s):
                hp, qb, kj = strip_info(s)
                e3 = e_of[(s // 3) * 3]
                for i in (0, 1):
                    nc.tensor.matmul(
                        ps_o[(hp, qb)][i],
                        v1s_all[hp][i][:, kj, :],
                        e3[:, s % 3, i * QBLK : (i + 1) * QBLK],
                        start=(kj == 0), stop=(kj == n_kj - 1),
                        skip_group_check=True,
                    )

            def emit_output(hp, qb):
                for i in (0, 1):
                    h = 2 * hp + i
                    ot_sb = outp.tile([DV1, QBLK], F32, tag="ot",
                                      name=f"ot_{h}_{qb}")
                    nc.vector.tensor_copy(ot_sb, ps_o[(hp, qb)][i])
                    nc.gpsimd.dma_start(out=o_d[h, qb], in_=ot_sb)
                del ps_o[(hp, qb)]

            def ensure_ps_o(s):
                hp, qb, kj = strip_info(s)
                if kj == 0:
                    ps_o[(hp, qb)] = [
                        opsum.tile([DV1, QBLK], F32, tag="o",
                                   name=f"ps_o_{hp}_{qb}_{i}")
                        for i in (0, 1)
                    ]

            def post_triple(strips):
                """mask + PV + epilogue for a triple of strips."""
                s0 = strips[0]
                merged = (
                    len(strips) == 3
                    and strips[0] % n_kj <= n_kj - 3
                )
                if merged:
                    emit_mask_triple(s0)
                else:
                    for t in strips:
                        emit_mask_strip(t)
                for t in strips:
                    hp, qb, kj = strip_info(t)
                    ensure_ps_o(t)
                    emit_pv(t)
                    if kj == n_kj - 1:
                        emit_output(hp, qb)
                    # prefetch two pairs ahead early in qb0 (pairs 0/1 are
                    # loaded in the prologue)
                    if hp + 2 < npairs and qb == 0 and kj == 2:
                        emit_pair_loads(hp + 2, nc.sync, nc.gpsimd)
                for t in strips:
                    e_of.pop(t, None)
                e_of.pop(s0, None)

            # group strips by psum slot: slots (0,1) -> merged exp, slot 2 ->
            # solo. QK+exp emission runs LAG groups ahead of mask/PV/epilogue
            # so the PE queue always has the next QK pair in front of PV work
            # that waits on the DVE.
            groups = []
            s = 0
            while s < n_strips:
                if s % 3 == 0 and s + 1 < n_strips:
                    groups.append((s, s + 1))
                    s += 2
                else:
                    groups.append((s,))
                    s += 1

            LAG = 2
            pending = []
            triple = []

            def post_ready(force=False):
                nonlocal triple
                while triple and (len(triple) >= 3 or force):
                    take = triple[:3]
                    triple = triple[3:]
                    post_triple(take)

            for g in groups:
                for t in g:
                    emit_qk(t)
                if len(g) == 2:
                    emit_exp_pair(g[0])
                elif g[0] % 3 == 2:
                    emit_exp_solo(g[0])
                else:
                    emit_exp_tail(g[0])
                pending.append(g)
                if len(pending) > LAG:
                    triple.extend(pending.pop(0))
                    post_ready()
            for g in pending:
                triple.extend(g)
                post_ready()
            post_ready(force=True)

    nc.compile()
    return nc


_NC_CACHE: dict = {}


def _get_nc(nheads, S, DK, scale):
    key = (nheads, S, DK, scale)
    if key not in _NC_CACHE:
        _NC_CACHE[key] = build_attention_nc(nheads, S, DK, scale)
    return _NC_CACHE[key]


def make_in_maps(queries, keys, values, d_k, mask):
    """Host-side sharding + layout prep. Returns (in_maps, shape_info)."""
    import ml_dtypes

    BF = ml_dtypes.bfloat16
    B, H, S, DK = queries.shape
    BH = B * H
    assert BH % N_CORES == 0
    hpc = BH // N_CORES
    npairs = hpc // 2
    CH = S // 128
    n_kj = S // 128
    NCHK = 4

    q = np.ascontiguousarray(queries.reshape(BH, S, DK)).astype(BF)
    k = np.ascontiguousarray(keys.reshape(BH, S, DK)).astype(BF)
    v = np.ascontiguousarray(values.reshape(BH, S, DK)).astype(BF)

    # qT/kT: [BH//2 pairs, NCHK, 128, S/NCHK] with head i of a pair on
    # partitions 64i..64i+63, DK-major, chunked along S.
    def to_pairT(x):
        # [BH, S, DK] -> [BH, DK, S] -> [BH//2, 2*DK, S] -> chunked
        xt = x.transpose(0, 2, 1)
        xt = xt.reshape(BH // 2, 2 * DK, NCHK, S // NCHK)
        return np.ascontiguousarray(xt.transpose(0, 2, 1, 3))

    qT = to_pairT(q)
    kT = to_pairT(k)

    # v1: [BH, 128, CH, DK+1] with ones column baked in.
    v1 = np.ones((BH, 128, CH, DK + 1), dtype=BF)
    v1[:, :, :, :DK] = v.reshape(BH, CH, 128, DK).transpose(0, 2, 1, 3)

    # maskT: [128, n_kj, S] bf16 keep-mask (1 - mask), kj-within-tile on
    # partitions.
    mT = (1 - mask.reshape(S, S)).astype(BF).T  # [kj, qi]
    mT = np.ascontiguousarray(
        mT.reshape(n_kj, 128, S).transpose(1, 0, 2))

    in_maps = [
        {
            "qT": qT[c * npairs : (c + 1) * npairs],
            "kT": kT[c * npairs : (c + 1) * npairs],
            "v1": v1[c * hpc : (c + 1) * hpc],
            "maskT": mT,
        }
        for c in range(N_CORES)
    ]
    return in_maps, (B, H, S, DK, hpc)


def kernel(queries, keys, values, d_k, mask):
    from concourse.bass_utils import run_bass_kernel_spmd

    in_maps, (B, H, S, DK, hpc) = make_in_maps(queries, keys, values, d_k,
                                               mask)
    scale = 1.0 / math.sqrt(float(d_k))
    nc = _get_nc(hpc, S, DK, scale)

    res = run_bass_kernel_spmd(nc, in_maps, core_ids=list(range(N_CORES)))
    outs = []
    for r in res.results:
        O = np.asarray(r["out"])            # [hpc, n_qblk, DK+1, QBLK]
        num = O[:, :, :DK, :]
        den = O[:, :, DK : DK + 1, :]
        o = (num / den).transpose(0, 1, 3, 2)   # [hpc, n_qblk, QBLK, DK]
        outs.append(o.reshape(hpc, S, DK))
    out = np.concatenate(outs, axis=0)
    return out.reshape(B, H, S, DK).astype(queries.dtype)


# revision 4
# speedup vs baseline: 1.0454x; 1.0246x over previous
"""Masked dot-product attention on 8 Trainium2 NeuronCores.

Strategy (per core): head-parallel sharding. B*H = 64 (batch, head) pairs are
split 8 per core; each core runs the full attention for its heads.

All layout transforms happen on the HOST (numpy) so the device only issues
plain contiguous DMAs:
  qT/kT:  [npairs, 4, 128, 512] bf16, head i of a pair on partitions
          64i..64i+63, DK-major, chunked along S so compute can start as soon
          as the first 128KB chunk lands.
  v1:     [nheads, 128, CH, 65] bf16, kj-within-chunk on partitions, with the
          ones column baked in (row dv=64 accumulates softmax denominators).
  maskT:  [128, n_kj, S] bf16 keep-mask (1-mask), kj-within-tile on
          partitions.

Per-head-pair pipeline (S=2048, DK=64), "S-transposed" layout so the PV
matmul needs no transpose of the huge exp matrix:
  S_T[kj, qi] = K @ Q^T        (PE, bf16, psum strips [128 kj, 2x512 qi];
                                the two heads' K=64 matmuls run CONCURRENTLY
                                in distinct PE row groups)
  E_T = exp(S_T / sqrt(dk))    (ScalarE; pair tile exp'd in ONE merged N=2048
                                ACTIVATE + a solo tile -> amortizes the fixed
                                per-instruction cost without cross-WARs)
  E_T *= maskT (keep 0/1)      (DVE tensor_tensor, bf16 2x mode; merged over
                                3 strips [128,3072] via a 4D mask AP when the
                                strips share (hp,qb))
  O_T[dv', qi] += V'[kj]^T E_T (PE accumulate over kj)
Epilogue: the unnormalized O_T[65, 512] (64 value rows + denominator row) is
copied PSUM->SBUF (DVE) and DMA'd out as-is; the softmax division and the
[dv, qi] -> [qi, dv] transpose happen on the HOST. This removes all PE
transposes and DVE reciprocal/multiply work from the device.

The QK/exp emission runs LAG groups ahead of the mask/PV/epilogue phase so
the PE queue always has the next QK pair in front of PV work that waits on
the DVE.
"""

import math

import numpy as np

import concourse.bass as bass
import concourse.mybir as mybir
import concourse.tile as tile
from concourse import bacc

F32 = mybir.dt.float32
BF16 = mybir.dt.bfloat16
AF = mybir.ActivationFunctionType
ALU = mybir.AluOpType

N_CORES = 8


def build_attention_nc(nheads: int, S: int, DK: int, scale: float) -> bass.Bass:
    nc = bacc.Bacc("TRN2", target_bir_lowering=False, debug=False,
                   num_devices=N_CORES)

    DV1 = DK + 1          # V plus a ones column for softmax denominators
    n_kj = S // 128       # kj tiles per head
    QBLK = 512            # qi span of one O_T accumulator
    n_qblk = S // QBLK
    CH = S // 128         # 128-row chunks along seq
    NCHK = 4              # q/k S-chunks per head-pair
    CHK = S // NCHK       # columns per chunk (512)
    npairs = nheads // 2
    assert nheads % 2 == 0

    qt_d = nc.dram_tensor("qT", [npairs, NCHK, 128, CHK], BF16,
                          kind="ExternalInput")
    kt_d = nc.dram_tensor("kT", [npairs, NCHK, 128, CHK], BF16,
                          kind="ExternalInput")
    v1_d = nc.dram_tensor("v1", [nheads, 128, CH, DV1], BF16,
                          kind="ExternalInput")
    m_d = nc.dram_tensor("maskT", [128, n_kj, S], BF16, kind="ExternalInput")
    o_d = nc.dram_tensor("out", [nheads, n_qblk, DV1, QBLK], F32,
                         kind="ExternalOutput")

    with tile.TileContext(nc) as tc:
        with (
            tc.tile_pool(name="maskp", bufs=1) as maskp,
            tc.tile_pool(name="qkT", bufs=3) as qkt,
            tc.tile_pool(name="vp", bufs=3) as vp,
            tc.tile_pool(name="ep", bufs=12) as ep,
            tc.tile_pool(name="outp", bufs=4) as outp,
            tc.tile_pool(name="ring", bufs=1, space="PSUM") as ringp,
            tc.tile_pool(name="opsum", bufs=2, space="PSUM") as opsum,
        ):
            # ---- per-pair inputs: plain chunked DMAs.
            qk_t = {}     # hp -> (q chunk tiles, k chunk tiles)
            v1s_all = {}  # hp -> [v1_h0, v1_h1]

            def emit_pair_loads(hp, eng, veng):
                kts, qts = [], []
                # k chunk 0 + q chunk 0 first: they unblock the first QKs.
                for c in range(NCHK):
                    kc = qkt.tile([128, CHK], BF16, tag=f"ktc{c}",
                                  name=f"ktc{c}_{hp}")
                    eng.dma_start(out=kc, in_=kt_d[hp, c])
                    kts.append(kc)
                    qc = qkt.tile([128, CHK], BF16, tag=f"qtc{c}",
                                  name=f"qtc{c}_{hp}")
                    eng.dma_start(out=qc, in_=qt_d[hp, c])
                    qts.append(qc)
                qk_t[hp] = (qts, kts)
                v1s = []
                for i in (0, 1):
                    v1 = vp.tile([128, CH, DV1], BF16, tag=f"v1_{i}",
                                 name=f"v1_{2 * hp + i}")
                    veng.dma_start(out=v1, in_=v1_d[2 * hp + i])
                    v1s.append(v1)
                v1s_all[hp] = v1s

            # pair 0 q/k goes FIRST so the compute pipeline starts ~3us in;
            # v1 + mask strips follow (first kj strips first - they are needed
            # soonest by the mask/PV phase).
            emit_pair_loads(0, nc.sync, nc.gpsimd)

            maskT = maskp.tile([128, n_kj, S], BF16, tag="maskT", name="maskT")
            for kt in range(n_kj):
                eng = nc.gpsimd if kt < n_kj // 2 else nc.sync
                eng.dma_start(out=maskT[:, kt, :], in_=m_d[:, kt, :])
            if npairs > 1:
                emit_pair_loads(1, nc.sync, nc.gpsimd)

            # ---- PSUM layout -----------------------------------------------
            # pairt: 2 strip slots for the merged-exp pairs (4 banks),
            # solot: 1 slot (2 banks) -> their WARs stay independent;
            # opsum: ps_o tiles share one rotating 2-buf tag (2 banks).
            pairt = ringp.tile([128, 2, 2 * QBLK], F32, tag="pair",
                               name="pairt")
            solot = ringp.tile([128, 2 * QBLK], F32, tag="solo", name="solot")

            # ---- main loop --------------------------------------------------
            n_strips = npairs * n_qblk * n_kj

            def strip_info(s):
                hp = s // (n_qblk * n_kj)
                qb = (s // n_kj) % n_qblk
                kj = s % n_kj
                return hp, qb, kj

            ps_o = {}     # (hp, qb) -> [ps_o_h0, ps_o_h1]
            e_of = {}     # s -> (e3_tile, slot)

            def emit_qk(s):
                hp, qb, kj = strip_info(s)
                qts, kts = qk_t[hp]
                slot = s % 3
                dst = pairt[:, slot, :] if slot < 2 else solot
                kc = kts[kj // (n_kj // NCHK)]
                k0 = (kj % (n_kj // NCHK)) * 128
                qc = qts[qb * QBLK // CHK]
                q0 = (qb * QBLK) % CHK
                for i in (0, 1):
                    nc.tensor.matmul(
                        dst[:, i * QBLK : (i + 1) * QBLK],
                        kc[64 * i : 64 * i + DK, k0 : k0 + 128],
                        qc[64 * i : 64 * i + DK, q0 : q0 + QBLK],
                        start=True, stop=True,
                    )

            def get_e3(s):
                """e3 tile shared by the 3 strips of s's triple."""
                t0 = (s // 3) * 3
                if t0 not in e_of:
                    e_of[t0] = ep.tile([128, 3, 2 * QBLK], BF16, tag="e3",
                                       name=f"e3_{t0}")
                return e_of[t0]

            def emit_exp_pair(s):
                # strips s (slot 0) and s+1 (slot 1) in one N=2048 ACTIVATE
                e3 = get_e3(s)
                nc.scalar.activation(e3[:, 0:2, :], pairt, AF.Exp, scale=scale)

            def emit_exp_solo(s):
                e3 = get_e3(s)
                nc.scalar.activation(e3[:, 2, :], solot, AF.Exp, scale=scale)

            def emit_exp_tail(s):
                # final unpaired strip landed on a pair slot
                e3 = get_e3(s)
                nc.scalar.activation(e3[:, s % 3, :], pairt[:, s % 3, :],
                                     AF.Exp, scale=scale)

            def emit_mask_strip(s):
                """fallback: mask one strip [128, 1024] with dup'd mask."""
                hp, qb, kj = strip_info(s)
                e3 = get_e3(s)
                ev = e3[:, s % 3, :]
                q0 = qb * QBLK
                msl = maskT[:, kj, q0 : q0 + QBLK]
                mdup = bass.AP(
                    tensor=msl.tensor, offset=msl.offset,
                    ap=[msl.ap[0], [0, 2], [1, QBLK]],
                )
                nc.vector.tensor_mul(ev, ev, mdup)

            def emit_mask_triple(s0):
                """merged: mask strips s0..s0+2 in one [128, 3072] DVE op."""
                hp, qb, kj = strip_info(s0)
                e3 = e_of[s0]
                q0 = qb * QBLK
                msl = maskT[:, kj, q0 : q0 + QBLK]
                m4 = bass.AP(
                    tensor=msl.tensor, offset=msl.offset,
                    ap=[msl.ap[0], [S, 3], [0, 2], [1, QBLK]],
                )
                nc.vector.tensor_mul(e3, e3, m4)

            def emit_pv(s):
                hp, qb, kj = strip_info(s)
                e3 = e_of[(s // 3) * 3]
                for i in (0, 1):
                    nc.tensor.matmul(
                        ps_o[(hp, qb)][i],
                        v1s_all[hp][i][:, kj, :],
                        e3[:, s % 3, i * QBLK : (i + 1) * QBLK],
                        start=(kj == 0), stop=(kj == n_kj - 1),
                        skip_group_check=True,
                    )

            def emit_output(hp, qb):
                for i in (0, 1):
                    h = 2 * hp + i
                    ot_sb = outp.tile([DV1, QBLK], F32, tag="ot",
                                      name=f"ot_{h}_{qb}")
                    nc.vector.tensor_copy(ot_sb, ps_o[(hp, qb)][i])
                    nc.gpsimd.dma_start(out=o_d[h, qb], in_=ot_sb)
                del ps_o[(hp, qb)]

            def ensure_ps_o(s):
                hp, qb, kj = strip_info(s)
                if kj == 0:
                    ps_o[(hp, qb)] = [
                        opsum.tile([DV1, QBLK], F32, tag="o",
                                   name=f"ps_o_{hp}_{qb}_{i}")
                        for i in (0, 1)
                    ]

            def post_triple(strips):
                """mask + PV + epilogue for a triple of strips."""
                s0 = strips[0]
                merged = (
                    len(strips) == 3
                    and strips[0] % n_kj <= n_kj - 3
                )
                if merged:
                    emit_mask_triple(s0)
                else:
                    for t in strips:
                        emit_mask_strip(t)
                for t in strips:
                    hp, qb, kj = strip_info(t)
                    ensure_ps_o(t)
                    emit_pv(t)
                    if kj == n_kj - 1:
                        emit_output(hp, qb)
                    # prefetch two pairs ahead early in qb0 (pairs 0/1 are
                    # loaded in the prologue)
                    if hp + 2 < npairs and qb == 0 and kj == 2:
                        emit_pair_loads(hp + 2, nc.sync, nc.gpsimd)
                for t in strips:
                    e_of.pop(t, None)
                e_of.pop(s0, None)

            # group strips by psum slot: slots (0,1) -> merged exp, slot 2 ->
            # solo. QK+exp emission runs LAG groups ahead of mask/PV/epilogue
            # so the PE queue always has the next QK pair in front of PV work
            # that waits on the DVE.
            groups = []
            s = 0
            while s < n_strips:
                if s % 3 == 0 and s + 1 < n_strips:
                    groups.append((s, s + 1))
                    s += 2
                else:
                    groups.append((s,))
                    s += 1

            # LAG control: a deep lag at startup keeps PV (which waits on the
            # mask DMAs) out of the in-order PE queue until the mask has
            # landed; at qb-block boundaries the first-kj PV waits on the ps_o
            # WAR (DVE copies), so those triples get extra slack too.
            LAG = 2
            START_THR = 13
            START_UNTIL = 24
            BOUND_EXTRA = 2
            pending = []
            triple = []

            def next_post_strip():
                if triple:
                    return triple[0]
                if pending:
                    return pending[0][0]
                return None

            def want_thr():
                s0 = next_post_strip()
                if s0 is None:
                    return LAG
                if s0 < START_UNTIL:
                    return START_THR
                if s0 % n_kj == 0:
                    return LAG + BOUND_EXTRA
                return LAG

            def post_ready(force=False):
                nonlocal triple
                while triple and (len(triple) >= 3 or force):
                    take = triple[:3]
                    triple = triple[3:]
                    post_triple(take)

            for g in groups:
                for t in g:
                    emit_qk(t)
                if len(g) == 2:
                    emit_exp_pair(g[0])
                elif g[0] % 3 == 2:
                    emit_exp_solo(g[0])
                else:
                    emit_exp_tail(g[0])
                pending.append(g)
                while len(pending) > want_thr():
                    triple.extend(pending.pop(0))
                    post_ready()
            for g in pending:
                triple.extend(g)
                post_ready()
            post_ready(force=True)

    nc.compile()
    return nc


_NC_CACHE: dict = {}


def _get_nc(nheads, S, DK, scale):
    key = (nheads, S, DK, scale)
    if key not in _NC_CACHE:
        _NC_CACHE[key] = build_attention_nc(nheads, S, DK, scale)
    return _NC_CACHE[key]


def make_in_maps(queries, keys, values, d_k, mask):
    """Host-side sharding + layout prep. Returns (in_maps, shape_info)."""
    import ml_dtypes

    BF = ml_dtypes.bfloat16
    B, H, S, DK = queries.shape
    BH = B * H
    assert BH % N_CORES == 0
    hpc = BH // N_CORES
    npairs = hpc // 2
    CH = S // 128
    n_kj = S // 128
    NCHK = 4

    q = np.ascontiguousarray(queries.reshape(BH, S, DK)).astype(BF)
    k = np.ascontiguousarray(keys.reshape(BH, S, DK)).astype(BF)
    v = np.ascontiguousarray(values.reshape(BH, S, DK)).astype(BF)

    # qT/kT: [BH//2 pairs, NCHK, 128, S/NCHK] with head i of a pair on
    # partitions 64i..64i+63, DK-major, chunked along S.
    def to_pairT(x):
        # [BH, S, DK] -> [BH, DK, S] -> [BH//2, 2*DK, S] -> chunked
        xt = x.transpose(0, 2, 1)
        xt = xt.reshape(BH // 2, 2 * DK, NCHK, S // NCHK)
        return np.ascontiguousarray(xt.transpose(0, 2, 1, 3))

    qT = to_pairT(q)
    kT = to_pairT(k)

    # v1: [BH, 128, CH, DK+1] with ones column baked in.
    v1 = np.ones((BH, 128, CH, DK + 1), dtype=BF)
    v1[:, :, :, :DK] = v.reshape(BH, CH, 128, DK).transpose(0, 2, 1, 3)

    # maskT: [128, n_kj, S] bf16 keep-mask (1 - mask), kj-within-tile on
    # partitions.
    mT = (1 - mask.reshape(S, S)).astype(BF).T  # [kj, qi]
    mT = np.ascontiguousarray(
        mT.reshape(n_kj, 128, S).transpose(1, 0, 2))

    in_maps = [
        {
            "qT": qT[c * npairs : (c + 1) * npairs],
            "kT": kT[c * npairs : (c + 1) * npairs],
            "v1": v1[c * hpc : (c + 1) * hpc],
            "maskT": mT,
        }
        for c in range(N_CORES)
    ]
    return in_maps, (B, H, S, DK, hpc)


def kernel(queries, keys, values, d_k, mask):
    from concourse.bass_utils import run_bass_kernel_spmd

    in_maps, (B, H, S, DK, hpc) = make_in_maps(queries, keys, values, d_k,
                                               mask)
    scale = 1.0 / math.sqrt(float(d_k))
    nc = _get_nc(hpc, S, DK, scale)

    res = run_bass_kernel_spmd(nc, in_maps, core_ids=list(range(N_CORES)))
    outs = []
    for r in res.results:
        O = np.asarray(r["out"])            # [hpc, n_qblk, DK+1, QBLK]
        num = O[:, :, :DK, :]
        den = O[:, :, DK : DK + 1, :]
        o = (num / den).transpose(0, 1, 3, 2)   # [hpc, n_qblk, QBLK, DK]
        outs.append(o.reshape(hpc, S, DK))
    out = np.concatenate(outs, axis=0)
    return out.reshape(B, H, S, DK).astype(queries.dtype)


# revision 8
# speedup vs baseline: 1.0580x; 1.0120x over previous
"""Masked dot-product attention on 8 Trainium2 NeuronCores.

Strategy (per core): head-parallel sharding. B*H = 64 (batch, head) pairs are
split 8 per core; each core runs the full attention for its heads.

All layout transforms happen on the HOST (numpy) so the device only issues
plain contiguous DMAs:
  qT/kT:  [npairs, 4, 128, 512] bf16, head i of a pair on partitions
          64i..64i+63, DK-major, chunked along S so compute can start as soon
          as the first 128KB chunk lands.
  v1:     [nheads, 128, CH, 65] bf16, kj-within-chunk on partitions, with the
          ones column baked in (row dv=64 accumulates softmax denominators).
  maskT:  [128, n_kj, S] bf16 keep-mask (1-mask), kj-within-tile on
          partitions.

Per-head-pair pipeline (S=2048, DK=64), "S-transposed" layout so the PV
matmul needs no transpose of the huge exp matrix:
  S_T[kj, qi] = K @ Q^T        (PE, bf16, psum strips [128 kj, 2x512 qi];
                                the two heads' K=64 matmuls run CONCURRENTLY
                                in distinct PE row groups)
  E_T = exp(S_T / sqrt(dk))    (ScalarE; pair tile exp'd in ONE merged N=2048
                                ACTIVATE + a solo tile -> amortizes the fixed
                                per-instruction cost without cross-WARs)
  E_T *= maskT (keep 0/1)      (DVE tensor_tensor, bf16 2x mode; merged over
                                3 strips [128,3072] via a 4D mask AP when the
                                strips share (hp,qb))
  O_T[dv', qi] += V'[kj]^T E_T (PE accumulate over kj)
Epilogue: the unnormalized O_T[65, 512] (64 value rows + denominator row) is
copied PSUM->SBUF (DVE) and DMA'd out as-is; the softmax division and the
[dv, qi] -> [qi, dv] transpose happen on the HOST. This removes all PE
transposes and DVE reciprocal/multiply work from the device.

The QK/exp emission runs LAG groups ahead of the mask/PV/epilogue phase so
the PE queue always has the next QK pair in front of PV work that waits on
the DVE.
"""

import math

import numpy as np

import concourse.bass as bass
import concourse.mybir as mybir
import concourse.tile as tile
from concourse import bacc

F32 = mybir.dt.float32
BF16 = mybir.dt.bfloat16
AF = mybir.ActivationFunctionType
ALU = mybir.AluOpType

N_CORES = 8


def build_attention_nc(nheads: int, S: int, DK: int, scale: float) -> bass.Bass:
    nc = bacc.Bacc("TRN2", target_bir_lowering=False, debug=False,
                   num_devices=N_CORES)

    DV1 = DK + 1          # V plus a ones column for softmax denominators
    n_kj = S // 128       # kj tiles per head
    QBLK = 512            # qi span of one O_T accumulator
    n_qblk = S // QBLK
    CH = S // 128         # 128-row chunks along seq
    NCHK = 4              # q/k S-chunks per head-pair
    CHK = S // NCHK       # columns per chunk (512)
    npairs = nheads // 2
    assert nheads % 2 == 0

    qt_d = nc.dram_tensor("qT", [npairs, NCHK, 128, CHK], BF16,
                          kind="ExternalInput")
    kt_d = nc.dram_tensor("kT", [npairs, NCHK, 128, CHK], BF16,
                          kind="ExternalInput")
    v1_d = nc.dram_tensor("v1", [nheads, 128, CH, DV1], BF16,
                          kind="ExternalInput")
    m_d = nc.dram_tensor("maskT", [128, n_kj, S], BF16, kind="ExternalInput")
    o_d = nc.dram_tensor("out", [nheads, n_qblk, DV1, QBLK], F32,
                         kind="ExternalOutput")

    with tile.TileContext(nc) as tc:
        with (
            tc.tile_pool(name="maskp", bufs=1) as maskp,
            tc.tile_pool(name="qkT", bufs=3) as qkt,
            tc.tile_pool(name="vp", bufs=3) as vp,
            tc.tile_pool(name="ep", bufs=12) as ep,
            tc.tile_pool(name="outp", bufs=4) as outp,
            tc.tile_pool(name="ring", bufs=1, space="PSUM") as ringp,
            tc.tile_pool(name="opsum", bufs=2, space="PSUM") as opsum,
        ):
            # ---- per-pair inputs: plain chunked DMAs.
            qk_t = {}     # hp -> (q chunk tiles, k chunk tiles)
            v1s_all = {}  # hp -> [v1_h0, v1_h1]

            def emit_pair_loads(hp, eng, veng):
                kts, qts = [], []
                # k chunk 0 + q chunk 0 first: they unblock the first QKs.
                for c in range(NCHK):
                    kc = qkt.tile([128, CHK], BF16, tag=f"ktc{c}",
                                  name=f"ktc{c}_{hp}")
                    eng.dma_start(out=kc, in_=kt_d[hp, c])
                    kts.append(kc)
                    qc = qkt.tile([128, CHK], BF16, tag=f"qtc{c}",
                                  name=f"qtc{c}_{hp}")
                    eng.dma_start(out=qc, in_=qt_d[hp, c])
                    qts.append(qc)
                qk_t[hp] = (qts, kts)
                v1s = []
                for i in (0, 1):
                    v1 = vp.tile([128, CH, DV1], BF16, tag=f"v1_{i}",
                                 name=f"v1_{2 * hp + i}")
                    veng.dma_start(out=v1, in_=v1_d[2 * hp + i])
                    v1s.append(v1)
                v1s_all[hp] = v1s

            # pair 0 q/k goes FIRST so the compute pipeline starts ~3us in;
            # v1 + mask strips follow (first kj strips first - they are needed
            # soonest by the mask/PV phase).
            emit_pair_loads(0, nc.sync, nc.gpsimd)

            # mask strips spread over all four engine queues so the full 8MB
            # lands by ~9us; low kt first (needed soonest by the PV phase).
            maskT = maskp.tile([128, n_kj, S], BF16, tag="maskT", name="maskT")
            for kt in range(n_kj):
                if kt < 5:
                    eng = nc.gpsimd
                elif kt < 11:
                    eng = nc.scalar
                else:
                    eng = nc.sync
                eng.dma_start(out=maskT[:, kt, :], in_=m_d[:, kt, :])
            if npairs > 1:
                emit_pair_loads(1, nc.sync, nc.gpsimd)

            # ---- PSUM layout -----------------------------------------------
            # pairt: 2 strip slots for the merged-exp pairs (4 banks),
            # solot: 1 slot (2 banks) -> their WARs stay independent;
            # opsum: ps_o tiles share one rotating 2-buf tag (2 banks).
            pairt = ringp.tile([128, 2, 2 * QBLK], F32, tag="pair",
                               name="pairt")
            solot = ringp.tile([128, 2 * QBLK], F32, tag="solo", name="solot")

            # ---- main loop --------------------------------------------------
            n_strips = npairs * n_qblk * n_kj

            def strip_info(s):
                hp = s // (n_qblk * n_kj)
                qb = (s // n_kj) % n_qblk
                kj = s % n_kj
                return hp, qb, kj

            ps_o = {}     # (hp, qb) -> [ps_o_h0, ps_o_h1]
            e_of = {}     # s -> (e3_tile, slot)

            def emit_qk(s):
                hp, qb, kj = strip_info(s)
                qts, kts = qk_t[hp]
                slot = s % 3
                dst = pairt[:, slot, :] if slot < 2 else solot
                kc = kts[kj // (n_kj // NCHK)]
                k0 = (kj % (n_kj // NCHK)) * 128
                qc = qts[qb * QBLK // CHK]
                q0 = (qb * QBLK) % CHK
                for i in (0, 1):
                    nc.tensor.matmul(
                        dst[:, i * QBLK : (i + 1) * QBLK],
                        kc[64 * i : 64 * i + DK, k0 : k0 + 128],
                        qc[64 * i : 64 * i + DK, q0 : q0 + QBLK],
                        start=True, stop=True,
                    )

            def get_e3(s):
                """e3 tile shared by the 3 strips of s's triple."""
                t0 = (s // 3) * 3
                if t0 not in e_of:
                    e_of[t0] = ep.tile([128, 3, 2 * QBLK], BF16, tag="e3",
                                       name=f"e3_{t0}")
                return e_of[t0]

            def emit_exp_pair(s):
                # strips s (slot 0) and s+1 (slot 1) in one N=2048 ACTIVATE
                e3 = get_e3(s)
                nc.scalar.activation(e3[:, 0:2, :], pairt, AF.Exp, scale=scale)

            def emit_exp_solo(s):
                e3 = get_e3(s)
                nc.scalar.activation(e3[:, 2, :], solot, AF.Exp, scale=scale)

            def emit_exp_tail(s):
                # final unpaired strip landed on a pair slot
                e3 = get_e3(s)
                nc.scalar.activation(e3[:, s % 3, :], pairt[:, s % 3, :],
                                     AF.Exp, scale=scale)

            def emit_mask_strip(s):
                """fallback: mask one strip [128, 1024] with dup'd mask."""
                hp, qb, kj = strip_info(s)
                e3 = get_e3(s)
                ev = e3[:, s % 3, :]
                q0 = qb * QBLK
                msl = maskT[:, kj, q0 : q0 + QBLK]
                mdup = bass.AP(
                    tensor=msl.tensor, offset=msl.offset,
                    ap=[msl.ap[0], [0, 2], [1, QBLK]],
                )
                nc.vector.tensor_mul(ev, ev, mdup)

            def emit_mask_triple(s0):
                """merged: mask strips s0..s0+2 in one [128, 3072] DVE op."""
                hp, qb, kj = strip_info(s0)
                e3 = e_of[s0]
                q0 = qb * QBLK
                msl = maskT[:, kj, q0 : q0 + QBLK]
                m4 = bass.AP(
                    tensor=msl.tensor, offset=msl.offset,
                    ap=[msl.ap[0], [S, 3], [0, 2], [1, QBLK]],
                )
                nc.vector.tensor_mul(e3, e3, m4)

            def emit_pv(s):
                hp, qb, kj = strip_info(s)
                e3 = e_of[(s // 3) * 3]
                for i in (0, 1):
                    nc.tensor.matmul(
                        ps_o[(hp, qb)][i],
                        v1s_all[hp][i][:, kj, :],
                        e3[:, s % 3, i * QBLK : (i + 1) * QBLK],
                        start=(kj == 0), stop=(kj == n_kj - 1),
                        skip_group_check=True,
                    )

            def emit_output(hp, qb):
                for i in (0, 1):
                    h = 2 * hp + i
                    ot_sb = outp.tile([DV1, QBLK], F32, tag="ot",
                                      name=f"ot_{h}_{qb}")
                    nc.vector.tensor_copy(ot_sb, ps_o[(hp, qb)][i])
                    nc.gpsimd.dma_start(out=o_d[h, qb], in_=ot_sb)
                del ps_o[(hp, qb)]

            def ensure_ps_o(s):
                hp, qb, kj = strip_info(s)
                if kj == 0:
                    ps_o[(hp, qb)] = [
                        opsum.tile([DV1, QBLK], F32, tag="o",
                                   name=f"ps_o_{hp}_{qb}_{i}")
                        for i in (0, 1)
                    ]

            def post_triple(strips):
                """mask + PV + epilogue for a triple of strips."""
                s0 = strips[0]
                merged = (
                    len(strips) == 3
                    and strips[0] % n_kj <= n_kj - 3
                )
                if merged:
                    emit_mask_triple(s0)
                else:
                    for t in strips:
                        emit_mask_strip(t)
                for t in strips:
                    hp, qb, kj = strip_info(t)
                    ensure_ps_o(t)
                    emit_pv(t)
                    if kj == n_kj - 1:
                        emit_output(hp, qb)
                    # prefetch two pairs ahead early in qb0 (pairs 0/1 are
                    # loaded in the prologue)
                    if hp + 2 < npairs and qb == 0 and kj == 2:
                        emit_pair_loads(hp + 2, nc.sync, nc.gpsimd)
                for t in strips:
                    e_of.pop(t, None)
                e_of.pop(s0, None)

            # group strips by psum slot: slots (0,1) -> merged exp, slot 2 ->
            # solo. QK+exp emission runs LAG groups ahead of mask/PV/epilogue
            # so the PE queue always has the next QK pair in front of PV work
            # that waits on the DVE.
            groups = []
            s = 0
            while s < n_strips:
                if s % 3 == 0 and s + 1 < n_strips:
                    groups.append((s, s + 1))
                    s += 2
                else:
                    groups.append((s,))
                    s += 1

            # LAG control: a deep lag at startup keeps PV (which waits on the
            # mask DMAs) out of the in-order PE queue until the mask has
            # landed; at qb-block boundaries the first-kj PV waits on the ps_o
            # WAR (DVE copies), so those triples get extra slack too.
            LAG = 2
            START_THR = 9
            START_UNTIL = 18
            BOUND_EXTRA = 2
            pending = []
            triple = []

            def next_post_strip():
                if triple:
                    return triple[0]
                if pending:
                    return pending[0][0]
                return None

            def want_thr():
                s0 = next_post_strip()
                if s0 is None:
                    return LAG
                if s0 < START_UNTIL:
                    return START_THR
                # next triple contains a kj==0 strip (its PV waits the ps_o
                # WAR on the previous block's DVE drain) -> extra slack
                if s0 % n_kj >= n_kj - 2 or s0 % n_kj == 0:
                    return LAG + BOUND_EXTRA
                return LAG

            def post_ready(force=False):
                nonlocal triple
                while triple and (len(triple) >= 3 or force):
                    take = triple[:3]
                    triple = triple[3:]
                    post_triple(take)

            for g in groups:
                for t in g:
                    emit_qk(t)
                if len(g) == 2:
                    emit_exp_pair(g[0])
                elif g[0] % 3 == 2:
                    emit_exp_solo(g[0])
                else:
                    emit_exp_tail(g[0])
                pending.append(g)
                while len(pending) > want_thr():
                    triple.extend(pending.pop(0))
                    post_ready()
            for g in pending:
                triple.extend(g)
                post_ready()
            post_ready(force=True)

    nc.compile()
    return nc


_NC_CACHE: dict = {}


def _get_nc(nheads, S, DK, scale):
    key = (nheads, S, DK, scale)
    if key not in _NC_CACHE:
        _NC_CACHE[key] = build_attention_nc(nheads, S, DK, scale)
    return _NC_CACHE[key]


def make_in_maps(queries, keys, values, d_k, mask):
    """Host-side sharding + layout prep. Returns (in_maps, shape_info)."""
    import ml_dtypes

    BF = ml_dtypes.bfloat16
    B, H, S, DK = queries.shape
    BH = B * H
    assert BH % N_CORES == 0
    hpc = BH // N_CORES
    npairs = hpc // 2
    CH = S // 128
    n_kj = S // 128
    NCHK = 4

    q = np.ascontiguousarray(queries.reshape(BH, S, DK)).astype(BF)
    k = np.ascontiguousarray(keys.reshape(BH, S, DK)).astype(BF)
    v = np.ascontiguousarray(values.reshape(BH, S, DK)).astype(BF)

    # qT/kT: [BH//2 pairs, NCHK, 128, S/NCHK] with head i of a pair on
    # partitions 64i..64i+63, DK-major, chunked along S.
    def to_pairT(x):
        # [BH, S, DK] -> [BH, DK, S] -> [BH//2, 2*DK, S] -> chunked
        xt = x.transpose(0, 2, 1)
        xt = xt.reshape(BH // 2, 2 * DK, NCHK, S // NCHK)
        return np.ascontiguousarray(xt.transpose(0, 2, 1, 3))

    qT = to_pairT(q)
    kT = to_pairT(k)

    # v1: [BH, 128, CH, DK+1] with ones column baked in.
    v1 = np.ones((BH, 128, CH, DK + 1), dtype=BF)
    v1[:, :, :, :DK] = v.reshape(BH, CH, 128, DK).transpose(0, 2, 1, 3)

    # maskT: [128, n_kj, S] bf16 keep-mask (1 - mask), kj-within-tile on
    # partitions.
    mT = (1 - mask.reshape(S, S)).astype(BF).T  # [kj, qi]
    mT = np.ascontiguousarray(
        mT.reshape(n_kj, 128, S).transpose(1, 0, 2))

    in_maps = [
        {
            "qT": qT[c * npairs : (c + 1) * npairs],
            "kT": kT[c * npairs : (c + 1) * npairs],
            "v1": v1[c * hpc : (c + 1) * hpc],
            "maskT": mT,
        }
        for c in range(N_CORES)
    ]
    return in_maps, (B, H, S, DK, hpc)


def kernel(queries, keys, values, d_k, mask):
    from concourse.bass_utils import run_bass_kernel_spmd

    in_maps, (B, H, S, DK, hpc) = make_in_maps(queries, keys, values, d_k,
                                               mask)
    scale = 1.0 / math.sqrt(float(d_k))
    nc = _get_nc(hpc, S, DK, scale)

    res = run_bass_kernel_spmd(nc, in_maps, core_ids=list(range(N_CORES)))
    outs = []
    for r in res.results:
        O = np.asarray(r["out"])            # [hpc, n_qblk, DK+1, QBLK]
        num = O[:, :, :DK, :]
        den = O[:, :, DK : DK + 1, :]
        o = (num / den).transpose(0, 1, 3, 2)   # [hpc, n_qblk, QBLK, DK]
        outs.append(o.reshape(hpc, S, DK))
    out = np.concatenate(outs, axis=0)
    return out.reshape(B, H, S, DK).astype(queries.dtype)


# revision 11
# speedup vs baseline: 1.0947x; 1.0348x over previous
"""Masked dot-product attention on 8 Trainium2 NeuronCores.

Strategy (per core): head-parallel sharding. B*H = 64 (batch, head) pairs are
split 8 per core; each core runs the full attention for its heads.

All layout transforms happen on the HOST (numpy) so the device only issues
plain contiguous DMAs:
  qT/kT:  [npairs, 4, 128, 512] bf16, head i of a pair on partitions
          64i..64i+63, DK-major, chunked along S so compute can start as soon
          as the first 128KB chunk lands.
  v1:     [nheads, 128, CH, 65] bf16, kj-within-chunk on partitions, with the
          ones column baked in (row dv=64 accumulates softmax denominators).
  maskT:  [128, n_kj, S] bf16 keep-mask (1-mask), kj-within-tile on
          partitions.

Per-head-pair pipeline (S=2048, DK=64), "S-transposed" layout so the PV
matmul needs no transpose of the huge exp matrix:
  S_T[kj, qi] = K @ Q^T        (PE, bf16, psum strips [128 kj, 2x512 qi];
                                the two heads' K=64 matmuls run CONCURRENTLY
                                in distinct PE row groups)
  E_T = exp(S_T / sqrt(dk))    (ScalarE; pair tile exp'd in ONE merged N=2048
                                ACTIVATE + a solo tile -> amortizes the fixed
                                per-instruction cost without cross-WARs)
  E_T *= maskT (keep 0/1)      (DVE tensor_tensor, bf16 2x mode; merged over
                                3 strips [128,3072] via a 4D mask AP when the
                                strips share (hp,qb))
  O_T[dv', qi] += V'[kj]^T E_T (PE accumulate over kj)
Epilogue: the unnormalized O_T[65, 512] (64 value rows + denominator row) is
copied PSUM->SBUF (DVE) and DMA'd out as-is; the softmax division and the
[dv, qi] -> [qi, dv] transpose happen on the HOST. This removes all PE
transposes and DVE reciprocal/multiply work from the device.

The QK/exp emission runs LAG groups ahead of the mask/PV/epilogue phase so
the PE queue always has the next QK pair in front of PV work that waits on
the DVE.
"""

import math

import numpy as np

import concourse.bass as bass
import concourse.mybir as mybir
import concourse.tile as tile
from concourse import bacc

F32 = mybir.dt.float32
BF16 = mybir.dt.bfloat16
AF = mybir.ActivationFunctionType
ALU = mybir.AluOpType

N_CORES = 8


def build_attention_nc(nheads: int, S: int, DK: int, scale: float) -> bass.Bass:
    nc = bacc.Bacc("TRN2", target_bir_lowering=False, debug=False,
                   num_devices=N_CORES)

    DV1 = DK + 1          # V plus a ones column for softmax denominators
    n_kj = S // 128       # kj tiles per head
    QBLK = 512            # qi span of one O_T accumulator
    n_qblk = S // QBLK
    CH = S // 128         # 128-row chunks along seq
    NCHK = 4              # q/k S-chunks per head-pair
    CHK = S // NCHK       # columns per chunk (512)
    npairs = nheads // 2
    assert nheads % 2 == 0

    qt_d = nc.dram_tensor("qT", [npairs, NCHK, 128, CHK], BF16,
                          kind="ExternalInput")
    kt_d = nc.dram_tensor("kT", [npairs, NCHK, 128, CHK], BF16,
                          kind="ExternalInput")
    v1_d = nc.dram_tensor("v1", [nheads, 128, CH, DV1], BF16,
                          kind="ExternalInput")
    m_d = nc.dram_tensor("maskT", [128, n_kj, S], BF16, kind="ExternalInput")
    o_d = nc.dram_tensor("out", [nheads, n_qblk, DV1, QBLK], F32,
                         kind="ExternalOutput")

    with tile.TileContext(nc) as tc:
        with (
            tc.tile_pool(name="maskp", bufs=1) as maskp,
            tc.tile_pool(name="qkT", bufs=3) as qkt,
            tc.tile_pool(name="vp", bufs=3) as vp,
            tc.tile_pool(name="ep", bufs=12) as ep,
            tc.tile_pool(name="outp", bufs=4) as outp,
            tc.tile_pool(name="ring", bufs=1, space="PSUM") as ringp,
            tc.tile_pool(name="opsum", bufs=2, space="PSUM") as opsum,
        ):
            # ---- per-pair inputs: plain chunked DMAs.
            qk_t = {}     # hp -> (q chunk tiles, k chunk tiles)
            v1s_all = {}  # hp -> [v1_h0, v1_h1]

            def emit_pair_loads(hp, eng, veng):
                kts, qts = [], []
                # k chunk 0 + q chunk 0 first: they unblock the first QKs.
                for c in range(NCHK):
                    kc = qkt.tile([128, CHK], BF16, tag=f"ktc{c}",
                                  name=f"ktc{c}_{hp}")
                    eng.dma_start(out=kc, in_=kt_d[hp, c])
                    kts.append(kc)
                    qc = qkt.tile([128, CHK], BF16, tag=f"qtc{c}",
                                  name=f"qtc{c}_{hp}")
                    eng.dma_start(out=qc, in_=qt_d[hp, c])
                    qts.append(qc)
                qk_t[hp] = (qts, kts)
                v1s = []
                for i in (0, 1):
                    v1 = vp.tile([128, CH, DV1], BF16, tag=f"v1_{i}",
                                 name=f"v1_{2 * hp + i}")
                    veng.dma_start(out=v1, in_=v1_d[2 * hp + i])
                    v1s.append(v1)
                v1s_all[hp] = v1s

            # pair 0 q/k goes FIRST so the compute pipeline starts ~3us in;
            # v1 + mask strips follow (first kj strips first - they are needed
            # soonest by the mask/PV phase).
            emit_pair_loads(0, nc.sync, nc.gpsimd)

            # mask strips spread over all four engine queues so the full 8MB
            # lands by ~9us; low kt first (needed soonest by the PV phase).
            maskT = maskp.tile([128, n_kj, S], BF16, tag="maskT", name="maskT")
            for kt in range(n_kj):
                eng = nc.gpsimd if kt < n_kj // 2 else nc.sync
                eng.dma_start(out=maskT[:, kt, :], in_=m_d[:, kt, :])
            if npairs > 1:
                emit_pair_loads(1, nc.sync, nc.gpsimd)

            # ---- PSUM layout -----------------------------------------------
            # pairt: 2 strip slots for the merged-exp pairs (4 banks),
            # solot: 1 slot (2 banks) -> their WARs stay independent;
            # opsum: ps_o tiles share one rotating 2-buf tag (2 banks).
            pairt = ringp.tile([128, 2, 2 * QBLK], F32, tag="pair",
                               name="pairt")
            solot = ringp.tile([128, 2 * QBLK], F32, tag="solo", name="solot")

            # ---- main loop --------------------------------------------------
            n_strips = npairs * n_qblk * n_kj

            def strip_info(s):
                hp = s // (n_qblk * n_kj)
                qb = (s // n_kj) % n_qblk
                kj = s % n_kj
                return hp, qb, kj

            ps_o = {}     # (hp, qb) -> [ps_o_h0, ps_o_h1]
            e_of = {}     # s -> (e3_tile, slot)

            def emit_qk(s):
                hp, qb, kj = strip_info(s)
                qts, kts = qk_t[hp]
                slot = s % 3
                dst = pairt[:, slot, :] if slot < 2 else solot
                kc = kts[kj // (n_kj // NCHK)]
                k0 = (kj % (n_kj // NCHK)) * 128
                qc = qts[qb * QBLK // CHK]
                q0 = (qb * QBLK) % CHK
                for i in (0, 1):
                    nc.tensor.matmul(
                        dst[:, i * QBLK : (i + 1) * QBLK],
                        kc[64 * i : 64 * i + DK, k0 : k0 + 128],
                        qc[64 * i : 64 * i + DK, q0 : q0 + QBLK],
                        start=True, stop=True,
                    )

            def get_e3(s):
                """e3 tile shared by the 3 strips of s's triple."""
                t0 = (s // 3) * 3
                if t0 not in e_of:
                    e_of[t0] = ep.tile([128, 3, 2 * QBLK], BF16, tag="e3",
                                       name=f"e3_{t0}")
                return e_of[t0]

            def emit_exp_pair(s):
                # strips s (slot 0) and s+1 (slot 1) in one N=2048 ACTIVATE
                e3 = get_e3(s)
                nc.scalar.activation(e3[:, 0:2, :], pairt, AF.Exp, scale=scale)

            def emit_exp_solo(s):
                e3 = get_e3(s)
                nc.scalar.activation(e3[:, 2, :], solot, AF.Exp, scale=scale)

            def emit_exp_tail(s):
                # final unpaired strip landed on a pair slot
                e3 = get_e3(s)
                nc.scalar.activation(e3[:, s % 3, :], pairt[:, s % 3, :],
                                     AF.Exp, scale=scale)

            def emit_mask_strip(s):
                """fallback: mask one strip [128, 1024] with dup'd mask."""
                hp, qb, kj = strip_info(s)
                e3 = get_e3(s)
                ev = e3[:, s % 3, :]
                q0 = qb * QBLK
                msl = maskT[:, kj, q0 : q0 + QBLK]
                mdup = bass.AP(
                    tensor=msl.tensor, offset=msl.offset,
                    ap=[msl.ap[0], [0, 2], [1, QBLK]],
                )
                nc.vector.tensor_mul(ev, ev, mdup)

            def emit_mask_triple(s0):
                """merged: mask strips s0..s0+2 in one [128, 3072] DVE op."""
                hp, qb, kj = strip_info(s0)
                e3 = e_of[s0]
                q0 = qb * QBLK
                msl = maskT[:, kj, q0 : q0 + QBLK]
                m4 = bass.AP(
                    tensor=msl.tensor, offset=msl.offset,
                    ap=[msl.ap[0], [S, 3], [0, 2], [1, QBLK]],
                )
                nc.vector.tensor_mul(e3, e3, m4)

            def emit_pv(s):
                hp, qb, kj = strip_info(s)
                e3 = e_of[(s // 3) * 3]
                for i in (0, 1):
                    nc.tensor.matmul(
                        ps_o[(hp, qb)][i],
                        v1s_all[hp][i][:, kj, :],
                        e3[:, s % 3, i * QBLK : (i + 1) * QBLK],
                        start=(kj == 0), stop=(kj == n_kj - 1),
                        skip_group_check=True,
                    )

            def emit_output(hp, qb):
                for i in (0, 1):
                    h = 2 * hp + i
                    ot_sb = outp.tile([DV1, QBLK], F32, tag="ot",
                                      name=f"ot_{h}_{qb}")
                    nc.vector.tensor_copy(ot_sb, ps_o[(hp, qb)][i])
                    nc.gpsimd.dma_start(out=o_d[h, qb], in_=ot_sb)
                del ps_o[(hp, qb)]

            def ensure_ps_o(s):
                hp, qb, kj = strip_info(s)
                if kj == 0:
                    ps_o[(hp, qb)] = [
                        opsum.tile([DV1, QBLK], F32, tag="o",
                                   name=f"ps_o_{hp}_{qb}_{i}")
                        for i in (0, 1)
                    ]

            def post_triple(strips):
                """mask + PV + epilogue for a triple of strips."""
                s0 = strips[0]
                merged = (
                    len(strips) == 3
                    and strips[0] % n_kj <= n_kj - 3
                )
                if merged:
                    emit_mask_triple(s0)
                else:
                    for t in strips:
                        emit_mask_strip(t)
                for t in strips:
                    hp, qb, kj = strip_info(t)
                    ensure_ps_o(t)
                    emit_pv(t)
                    if kj == n_kj - 1:
                        emit_output(hp, qb)
                    # prefetch two pairs ahead early in qb0 (pairs 0/1 are
                    # loaded in the prologue)
                    if hp + 2 < npairs and qb == 0 and kj == 2:
                        emit_pair_loads(hp + 2, nc.sync, nc.gpsimd)

            # group strips by psum slot: slots (0,1) -> merged exp, slot 2 ->
            # solo. QK+exp emission runs LAG groups ahead of mask/PV/epilogue
            # so the PE queue always has the next QK pair in front of PV work
            # that waits on the DVE.
            groups = []
            s = 0
            while s < n_strips:
                if s % 3 == 0 and s + 1 < n_strips:
                    groups.append((s, s + 1))
                    s += 2
                else:
                    groups.append((s,))
                    s += 1

            # LAG control: a deep lag at startup keeps PV (which waits on the
            # mask DMAs) out of the in-order PE queue until the mask has
            # landed; at qb-block boundaries the first-kj PV waits on the ps_o
            # WAR (DVE copies), so those triples get extra slack too.
            LAG = 2
            START_THR = 12
            START_UNTIL = 24
            BOUND_EXTRA = 2
            COOLDOWN = 2
            pending = []
            postq = []
            cooldown = 0

            def next_post_strip():
                if postq:
                    return postq[0]
                if pending:
                    return pending[0][0]
                return None

            def want_thr():
                s0 = next_post_strip()
                if s0 is None:
                    return LAG
                if s0 < START_UNTIL:
                    return START_THR
                # next triple contains a kj==0 strip (its PV waits the ps_o
                # WAR on the previous block's DVE drain) -> extra slack
                if s0 % n_kj >= n_kj - 2 or s0 % n_kj == 0:
                    return LAG + BOUND_EXTRA
                return LAG

            def post_ready(force=False):
                """Post queued strips. Triples that straddle a qb-block
                boundary are posted strip-by-strip with a cooldown before the
                kj==0 strip, so QK subgroups land between the epilogue drain
                and the next block's first PV in the in-order PE queue."""
                nonlocal postq, cooldown
                while postq:
                    t0 = (postq[0] // 3) * 3
                    crossing = t0 % n_kj >= n_kj - 2
                    if crossing:
                        s0 = postq[0]
                        if s0 % n_kj == 0 and cooldown > 0 and not force:
                            break
                        postq.pop(0)
                        post_triple([s0])
                        if s0 % n_kj == n_kj - 1:
                            cooldown = COOLDOWN + 1
                    elif len(postq) >= 3:
                        take, postq = postq[:3], postq[3:]
                        post_triple(take)
                    elif force:
                        take, postq = postq[:], []
                        post_triple(take)
                    else:
                        break

            for g in groups:
                for t in g:
                    emit_qk(t)
                if len(g) == 2:
                    emit_exp_pair(g[0])
                elif g[0] % 3 == 2:
                    emit_exp_solo(g[0])
                else:
                    emit_exp_tail(g[0])
                pending.append(g)
                if cooldown > 0:
                    cooldown -= 1
                post_ready()
                while len(pending) > want_thr():
                    postq.extend(pending.pop(0))
                    post_ready()
            while pending:
                postq.extend(pending.pop(0))
            post_ready(force=True)

    nc.compile()
    return nc


_NC_CACHE: dict = {}


def _get_nc(nheads, S, DK, scale):
    key = (nheads, S, DK, scale)
    if key not in _NC_CACHE:
        _NC_CACHE[key] = build_attention_nc(nheads, S, DK, scale)
    return _NC_CACHE[key]


def make_in_maps(queries, keys, values, d_k, mask):
    """Host-side sharding + layout prep. Returns (in_maps, shape_info)."""
    import ml_dtypes

    BF = ml_dtypes.bfloat16
    B, H, S, DK = queries.shape
    BH = B * H
    assert BH % N_CORES == 0
    hpc = BH // N_CORES
    npairs = hpc // 2
    CH = S // 128
    n_kj = S // 128
    NCHK = 4

    q = np.ascontiguousarray(queries.reshape(BH, S, DK)).astype(BF)
    k = np.ascontiguousarray(keys.reshape(BH, S, DK)).astype(BF)
    v = np.ascontiguousarray(values.reshape(BH, S, DK)).astype(BF)

    # qT/kT: [BH//2 pairs, NCHK, 128, S/NCHK] with head i of a pair on
    # partitions 64i..64i+63, DK-major, chunked along S.
    def to_pairT(x):
        # [BH, S, DK] -> [BH, DK, S] -> [BH//2, 2*DK, S] -> chunked
        xt = x.transpose(0, 2, 1)
        xt = xt.reshape(BH // 2, 2 * DK, NCHK, S // NCHK)
        return np.ascontiguousarray(xt.transpose(0, 2, 1, 3))

    qT = to_pairT(q)
    kT = to_pairT(k)

    # v1: [BH, 128, CH, DK+1] with ones column baked in.
    v1 = np.ones((BH, 128, CH, DK + 1), dtype=BF)
    v1[:, :, :, :DK] = v.reshape(BH, CH, 128, DK).transpose(0, 2, 1, 3)

    # maskT: [128, n_kj, S] bf16 keep-mask (1 - mask), kj-within-tile on
    # partitions.
    mT = (1 - mask.reshape(S, S)).astype(BF).T  # [kj, qi]
    mT = np.ascontiguousarray(
        mT.reshape(n_kj, 128, S).transpose(1, 0, 2))

    in_maps = [
        {
            "qT": qT[c * npairs : (c + 1) * npairs],
            "kT": kT[c * npairs : (c + 1) * npairs],
            "v1": v1[c * hpc : (c + 1) * hpc],
            "maskT": mT,
        }
        for c in range(N_CORES)
    ]
    return in_maps, (B, H, S, DK, hpc)


def kernel(queries, keys, values, d_k, mask):
    from concourse.bass_utils import run_bass_kernel_spmd

    in_maps, (B, H, S, DK, hpc) = make_in_maps(queries, keys, values, d_k,
                                               mask)
    scale = 1.0 / math.sqrt(float(d_k))
    nc = _get_nc(hpc, S, DK, scale)

    res = run_bass_kernel_spmd(nc, in_maps, core_ids=list(range(N_CORES)))
    outs = []
    for r in res.results:
        O = np.asarray(r["out"])            # [hpc, n_qblk, DK+1, QBLK]
        num = O[:, :, :DK, :]
        den = O[:, :, DK : DK + 1, :]
        o = (num / den).transpose(0, 1, 3, 2)   # [hpc, n_qblk, QBLK, DK]
        outs.append(o.reshape(hpc, S, DK))
    out = np.concatenate(outs, axis=0)
    return out.reshape(B, H, S, DK).astype(queries.dtype)


# revision 13
# speedup vs baseline: 1.1169x; 1.0203x over previous
"""Masked dot-product attention on 8 Trainium2 NeuronCores.

Strategy (per core): head-parallel sharding. B*H = 64 (batch, head) pairs are
split 8 per core; each core runs the full attention for its heads.

All layout transforms happen on the HOST (numpy) so the device only issues
plain contiguous DMAs:
  qT/kT:  [npairs, 4, 128, 512] bf16, head i of a pair on partitions
          64i..64i+63, DK-major, chunked along S so compute can start as soon
          as the first 128KB chunk lands.
  v1:     [nheads, 128, CH, 65] bf16, kj-within-chunk on partitions, with the
          ones column baked in (row dv=64 accumulates softmax denominators).
  maskT:  [128, n_kj, S] bf16 keep-mask (1-mask), kj-within-tile on
          partitions.

Per-head-pair pipeline (S=2048, DK=64), "S-transposed" layout so the PV
matmul needs no transpose of the huge exp matrix:
  S_T[kj, qi] = K @ Q^T        (PE, bf16, psum strips [128 kj, 2x512 qi];
                                the two heads' K=64 matmuls run CONCURRENTLY
                                in distinct PE row groups)
  E_T = exp(S_T / sqrt(dk))    (ScalarE; pair tile exp'd in ONE merged N=2048
                                ACTIVATE + a solo tile -> amortizes the fixed
                                per-instruction cost without cross-WARs)
  E_T *= maskT (keep 0/1)      (DVE tensor_tensor, bf16 2x mode; merged over
                                3 strips [128,3072] via a 4D mask AP when the
                                strips share (hp,qb))
  O_T[dv', qi] += V'[kj]^T E_T (PE accumulate over kj)
Epilogue: the unnormalized O_T[65, 512] (64 value rows + denominator row) is
copied PSUM->SBUF (DVE) and DMA'd out as-is; the softmax division and the
[dv, qi] -> [qi, dv] transpose happen on the HOST. This removes all PE
transposes and DVE reciprocal/multiply work from the device.

The QK/exp emission runs LAG groups ahead of the mask/PV/epilogue phase so
the PE queue always has the next QK pair in front of PV work that waits on
the DVE.
"""

import math

import numpy as np

import concourse.bass as bass
import concourse.mybir as mybir
import concourse.tile as tile
from concourse import bacc

F32 = mybir.dt.float32
BF16 = mybir.dt.bfloat16
AF = mybir.ActivationFunctionType
ALU = mybir.AluOpType

N_CORES = 8


def build_attention_nc(nheads: int, S: int, DK: int, scale: float) -> bass.Bass:
    nc = bacc.Bacc("TRN2", target_bir_lowering=False, debug=False,
                   num_devices=N_CORES)

    DV1 = DK + 1          # V plus a ones column for softmax denominators
    n_kj = S // 128       # kj tiles per head
    QBLK = 512            # qi span of one O_T accumulator
    n_qblk = S // QBLK
    CH = S // 128         # 128-row chunks along seq
    NCHK = 4              # q/k S-chunks per head-pair
    CHK = S // NCHK       # columns per chunk (512)
    npairs = nheads // 2
    assert nheads % 2 == 0

    qt_d = nc.dram_tensor("qT", [npairs, NCHK, 128, CHK], BF16,
                          kind="ExternalInput")
    kt_d = nc.dram_tensor("kT", [npairs, NCHK, 128, CHK], BF16,
                          kind="ExternalInput")
    v1_d = nc.dram_tensor("v1", [nheads, 128, CH, DV1], BF16,
                          kind="ExternalInput")
    m_d = nc.dram_tensor("maskT", [128, n_kj, S], BF16, kind="ExternalInput")
    o_d = nc.dram_tensor("out", [nheads, n_qblk, DV1, QBLK], F32,
                         kind="ExternalOutput")

    with tile.TileContext(nc) as tc:
        with (
            tc.tile_pool(name="maskp", bufs=1) as maskp,
            tc.tile_pool(name="qkT", bufs=3) as qkt,
            tc.tile_pool(name="vp", bufs=3) as vp,
            tc.tile_pool(name="ep", bufs=12) as ep,
            tc.tile_pool(name="outp", bufs=4) as outp,
            tc.tile_pool(name="ring", bufs=1, space="PSUM") as ringp,
            tc.tile_pool(name="opsum", bufs=2, space="PSUM") as opsum,
        ):
            # ---- per-pair inputs: plain chunked DMAs.
            qk_t = {}     # hp -> (q chunk tiles, k chunk tiles)
            v1s_all = {}  # hp -> [v1_h0, v1_h1]

            def emit_pair_loads(hp, eng, veng):
                kts, qts = [], []
                # k chunk 0 + q chunk 0 first: they unblock the first QKs.
                for c in range(NCHK):
                    kc = qkt.tile([128, CHK], BF16, tag=f"ktc{c}",
                                  name=f"ktc{c}_{hp}")
                    eng.dma_start(out=kc, in_=kt_d[hp, c])
                    kts.append(kc)
                    qc = qkt.tile([128, CHK], BF16, tag=f"qtc{c}",
                                  name=f"qtc{c}_{hp}")
                    eng.dma_start(out=qc, in_=qt_d[hp, c])
                    qts.append(qc)
                qk_t[hp] = (qts, kts)
                v1s = []
                for i in (0, 1):
                    v1 = vp.tile([128, CH, DV1], BF16, tag=f"v1_{i}",
                                 name=f"v1_{2 * hp + i}")
                    veng.dma_start(out=v1, in_=v1_d[2 * hp + i])
                    v1s.append(v1)
                v1s_all[hp] = v1s

            # Prologue DMA schedule. The scalar queue issues the three chunks
            # the first QK strips need (it blocks the ACT queue only at
            # t~1us, long before the first exp). sync then prioritizes the
            # rest of pair 0 followed by the mask half (the PV phase gates on
            # the whole mask tile landing); v1 rides gpsimd first.
            kts0, qts0 = [], []
            for c in range(NCHK):
                kts0.append(qkt.tile([128, CHK], BF16, tag=f"ktc{c}",
                                     name=f"ktc{c}_0"))
                qts0.append(qkt.tile([128, CHK], BF16, tag=f"qtc{c}",
                                     name=f"qtc{c}_0"))
            qk_t[0] = (qts0, kts0)
            nc.scalar.dma_start(out=kts0[0], in_=kt_d[0, 0])
            nc.scalar.dma_start(out=qts0[0], in_=qt_d[0, 0])
            nc.scalar.dma_start(out=qts0[1], in_=qt_d[0, 1])
            for c in (1, 2, 3):
                nc.sync.dma_start(out=kts0[c], in_=kt_d[0, c])
            v1s0 = []
            for i in (0, 1):
                v1 = vp.tile([128, CH, DV1], BF16, tag=f"v1_{i}",
                             name=f"v1_{i}")
                nc.gpsimd.dma_start(out=v1, in_=v1_d[i])
                v1s0.append(v1)
            v1s_all[0] = v1s0

            maskT = maskp.tile([128, n_kj, S], BF16, tag="maskT", name="maskT")
            for kt in range(n_kj):
                eng = nc.gpsimd if kt < n_kj // 2 else nc.sync
                eng.dma_start(out=maskT[:, kt, :], in_=m_d[:, kt, :])
            for c in (2, 3):
                nc.sync.dma_start(out=qts0[c], in_=qt_d[0, c])
            if npairs > 1:
                emit_pair_loads(1, nc.sync, nc.gpsimd)

            # ---- PSUM layout -----------------------------------------------
            # pairt: 2 strip slots for the merged-exp pairs (4 banks),
            # solot: 1 slot (2 banks) -> their WARs stay independent;
            # opsum: ps_o tiles share one rotating 2-buf tag (2 banks).
            pairt = ringp.tile([128, 2, 2 * QBLK], F32, tag="pair",
                               name="pairt")
            solot = ringp.tile([128, 2 * QBLK], F32, tag="solo", name="solot")

            # ---- main loop --------------------------------------------------
            n_strips = npairs * n_qblk * n_kj

            def strip_info(s):
                hp = s // (n_qblk * n_kj)
                qb = (s // n_kj) % n_qblk
                kj = s % n_kj
                return hp, qb, kj

            ps_o = {}     # (hp, qb) -> [ps_o_h0, ps_o_h1]
            e_of = {}     # s -> (e3_tile, slot)

            def emit_qk(s):
                hp, qb, kj = strip_info(s)
                qts, kts = qk_t[hp]
                slot = s % 3
                dst = pairt[:, slot, :] if slot < 2 else solot
                kc = kts[kj // (n_kj // NCHK)]
                k0 = (kj % (n_kj // NCHK)) * 128
                qc = qts[qb * QBLK // CHK]
                q0 = (qb * QBLK) % CHK
                for i in (0, 1):
                    nc.tensor.matmul(
                        dst[:, i * QBLK : (i + 1) * QBLK],
                        kc[64 * i : 64 * i + DK, k0 : k0 + 128],
                        qc[64 * i : 64 * i + DK, q0 : q0 + QBLK],
                        start=True, stop=True,
                    )

            def get_e3(s):
                """e3 tile shared by the 3 strips of s's triple."""
                t0 = (s // 3) * 3
                if t0 not in e_of:
                    e_of[t0] = ep.tile([128, 3, 2 * QBLK], BF16, tag="e3",
                                       name=f"e3_{t0}")
                return e_of[t0]

            def emit_exp_pair(s):
                # strips s (slot 0) and s+1 (slot 1) in one N=2048 ACTIVATE
                e3 = get_e3(s)
                nc.scalar.activation(e3[:, 0:2, :], pairt, AF.Exp, scale=scale)

            def emit_exp_solo(s):
                e3 = get_e3(s)
                nc.scalar.activation(e3[:, 2, :], solot, AF.Exp, scale=scale)

            def emit_exp_tail(s):
                # final unpaired strip landed on a pair slot
                e3 = get_e3(s)
                nc.scalar.activation(e3[:, s % 3, :], pairt[:, s % 3, :],
                                     AF.Exp, scale=scale)

            def emit_mask_strip(s):
                """fallback: mask one strip [128, 1024] with dup'd mask."""
                hp, qb, kj = strip_info(s)
                e3 = get_e3(s)
                ev = e3[:, s % 3, :]
                q0 = qb * QBLK
                msl = maskT[:, kj, q0 : q0 + QBLK]
                mdup = bass.AP(
                    tensor=msl.tensor, offset=msl.offset,
                    ap=[msl.ap[0], [0, 2], [1, QBLK]],
                )
                nc.vector.tensor_mul(ev, ev, mdup)

            def emit_mask_triple(s0):
                """merged: mask strips s0..s0+2 in one [128, 3072] DVE op."""
                hp, qb, kj = strip_info(s0)
                e3 = e_of[s0]
                q0 = qb * QBLK
                msl = maskT[:, kj, q0 : q0 + QBLK]
                m4 = bass.AP(
                    tensor=msl.tensor, offset=msl.offset,
                    ap=[msl.ap[0], [S, 3], [0, 2], [1, QBLK]],
                )
                nc.vector.tensor_mul(e3, e3, m4)

            def emit_pv(s):
                hp, qb, kj = strip_info(s)
                e3 = e_of[(s // 3) * 3]
                for i in (0, 1):
                    nc.tensor.matmul(
                        ps_o[(hp, qb)][i],
                        v1s_all[hp][i][:, kj, :],
                        e3[:, s % 3, i * QBLK : (i + 1) * QBLK],
                        start=(kj == 0), stop=(kj == n_kj - 1),
                        skip_group_check=True,
                    )

            def emit_output(hp, qb):
                for i in (0, 1):
                    h = 2 * hp + i
                    ot_sb = outp.tile([DV1, QBLK], F32, tag="ot",
                                      name=f"ot_{h}_{qb}")
                    nc.vector.tensor_copy(ot_sb, ps_o[(hp, qb)][i])
                    nc.gpsimd.dma_start(out=o_d[h, qb], in_=ot_sb)
                del ps_o[(hp, qb)]

            def ensure_ps_o(s):
                hp, qb, kj = strip_info(s)
                if kj == 0:
                    ps_o[(hp, qb)] = [
                        opsum.tile([DV1, QBLK], F32, tag="o",
                                   name=f"ps_o_{hp}_{qb}_{i}")
                        for i in (0, 1)
                    ]

            def post_triple(strips):
                """mask + PV + epilogue for a triple of strips."""
                s0 = strips[0]
                merged = (
                    len(strips) == 3
                    and strips[0] % n_kj <= n_kj - 3
                )
                if merged:
                    emit_mask_triple(s0)
                else:
                    for t in strips:
                        emit_mask_strip(t)
                for t in strips:
                    hp, qb, kj = strip_info(t)
                    ensure_ps_o(t)
                    emit_pv(t)
                    if kj == n_kj - 1:
                        emit_output(hp, qb)
                    # prefetch two pairs ahead early in qb0 (pairs 0/1 are
                    # loaded in the prologue)
                    if hp + 2 < npairs and qb == 0 and kj == 2:
                        emit_pair_loads(hp + 2, nc.sync, nc.gpsimd)

            # group strips by psum slot: slots (0,1) -> merged exp, slot 2 ->
            # solo. QK+exp emission runs LAG groups ahead of mask/PV/epilogue
            # so the PE queue always has the next QK pair in front of PV work
            # that waits on the DVE.
            groups = []
            s = 0
            while s < n_strips:
                if s % 3 == 0 and s + 1 < n_strips:
                    groups.append((s, s + 1))
                    s += 2
                else:
                    groups.append((s,))
                    s += 1

            # LAG control: a deep lag at startup keeps PV (which waits on the
            # mask DMAs) out of the in-order PE queue until the mask has
            # landed; at qb-block boundaries the first-kj PV waits on the ps_o
            # WAR (DVE copies), so those triples get extra slack too.
            LAG = 2
            START_THR = 12
            START_UNTIL = 24
            BOUND_EXTRA = 3
            COOLDOWN = 3
            pending = []
            postq = []
            cooldown = 0

            def next_post_strip():
                if postq:
                    return postq[0]
                if pending:
                    return pending[0][0]
                return None

            def want_thr():
                s0 = next_post_strip()
                if s0 is None:
                    return LAG
                if s0 < START_UNTIL:
                    return START_THR
                # next triple contains a kj==0 strip (its PV waits the ps_o
                # WAR on the previous block's DVE drain) -> extra slack
                if s0 % n_kj >= n_kj - 2 or s0 % n_kj == 0:
                    return LAG + BOUND_EXTRA
                return LAG

            def post_ready(force=False):
                """Post queued strips. Triples that straddle a qb-block
                boundary are posted strip-by-strip with a cooldown before the
                kj==0 strip, so QK subgroups land between the epilogue drain
                and the next block's first PV in the in-order PE queue."""
                nonlocal postq, cooldown
                while postq:
                    t0 = (postq[0] // 3) * 3
                    crossing = t0 % n_kj >= n_kj - 2
                    if crossing:
                        s0 = postq[0]
                        if s0 % n_kj == 0 and cooldown > 0 and not force:
                            break
                        postq.pop(0)
                        post_triple([s0])
                        if s0 % n_kj == n_kj - 1:
                            cooldown = COOLDOWN + 1
                    elif len(postq) >= 3:
                        take, postq = postq[:3], postq[3:]
                        post_triple(take)
                    elif force:
                        take, postq = postq[:], []
                        post_triple(take)
                    else:
                        break

            for g in groups:
                for t in g:
                    emit_qk(t)
                if len(g) == 2:
                    emit_exp_pair(g[0])
                elif g[0] % 3 == 2:
                    emit_exp_solo(g[0])
                else:
                    emit_exp_tail(g[0])
                pending.append(g)
                if cooldown > 0:
                    cooldown -= 1
                post_ready()
                while len(pending) > want_thr():
                    postq.extend(pending.pop(0))
                    post_ready()
            while pending:
                postq.extend(pending.pop(0))
            post_ready(force=True)

    nc.compile()
    return nc


_NC_CACHE: dict = {}


def _get_nc(nheads, S, DK, scale):
    key = (nheads, S, DK, scale)
    if key not in _NC_CACHE:
        _NC_CACHE[key] = build_attention_nc(nheads, S, DK, scale)
    return _NC_CACHE[key]


def make_in_maps(queries, keys, values, d_k, mask):
    """Host-side sharding + layout prep. Returns (in_maps, shape_info)."""
    import ml_dtypes

    BF = ml_dtypes.bfloat16
    B, H, S, DK = queries.shape
    BH = B * H
    assert BH % N_CORES == 0
    hpc = BH // N_CORES
    npairs = hpc // 2
    CH = S // 128
    n_kj = S // 128
    NCHK = 4

    q = np.ascontiguousarray(queries.reshape(BH, S, DK)).astype(BF)
    k = np.ascontiguousarray(keys.reshape(BH, S, DK)).astype(BF)
    v = np.ascontiguousarray(values.reshape(BH, S, DK)).astype(BF)

    # qT/kT: [BH//2 pairs, NCHK, 128, S/NCHK] with head i of a pair on
    # partitions 64i..64i+63, DK-major, chunked along S.
    def to_pairT(x):
        # [BH, S, DK] -> [BH, DK, S] -> [BH//2, 2*DK, S] -> chunked
        xt = x.transpose(0, 2, 1)
        xt = xt.reshape(BH // 2, 2 * DK, NCHK, S // NCHK)
        return np.ascontiguousarray(xt.transpose(0, 2, 1, 3))

    qT = to_pairT(q)
    kT = to_pairT(k)

    # v1: [BH, 128, CH, DK+1] with ones column baked in.
    v1 = np.ones((BH, 128, CH, DK + 1), dtype=BF)
    v1[:, :, :, :DK] = v.reshape(BH, CH, 128, DK).transpose(0, 2, 1, 3)

    # maskT: [128, n_kj, S] bf16 keep-mask (1 - mask), kj-within-tile on
    # partitions.
    mT = (1 - mask.reshape(S, S)).astype(BF).T  # [kj, qi]
    mT = np.ascontiguousarray(
        mT.reshape(n_kj, 128, S).transpose(1, 0, 2))

    in_maps = [
        {
            "qT": qT[c * npairs : (c + 1) * npairs],
            "kT": kT[c * npairs : (c + 1) * npairs],
            "v1": v1[c * hpc : (c + 1) * hpc],
            "maskT": mT,
        }
        for c in range(N_CORES)
    ]
    return in_maps, (B, H, S, DK, hpc)


def kernel(queries, keys, values, d_k, mask):
    from concourse.bass_utils import run_bass_kernel_spmd

    in_maps, (B, H, S, DK, hpc) = make_in_maps(queries, keys, values, d_k,
                                               mask)
    scale = 1.0 / math.sqrt(float(d_k))
    nc = _get_nc(hpc, S, DK, scale)

    res = run_bass_kernel_spmd(nc, in_maps, core_ids=list(range(N_CORES)))
    outs = []
    for r in res.results:
        O = np.asarray(r["out"])            # [hpc, n_qblk, DK+1, QBLK]
        num = O[:, :, :DK, :]
        den = O[:, :, DK : DK + 1, :]
        o = (num / den).transpose(0, 1, 3, 2)   # [hpc, n_qblk, QBLK, DK]
        outs.append(o.reshape(hpc, S, DK))
    out = np.concatenate(outs, axis=0)
    return out.reshape(B, H, S, DK).astype(queries.dtype)
